# revision 3
# baseline (speedup 1.0000x reference)
"""Multi-head attention (B=2, S=2048, D=1024, H=16, dk=dv=64) on 8 TRN2 NeuronCores.

Sharding: core c -> (batch b = c//4, head-group g = c%4, 4 heads each).
Each core computes q/k/v projections for its 4 heads (weight-column shard),
attention over its batch, and a partial output projection over its 256
channels (weight-row shard of Wo).  The host sums the 4 partial outputs per
batch at unshard time (the "all-reduce after the output projection").

v2 design notes (vs the fp32r baseline):
  * All matmul operands are bf16 (halves DMA traffic and SBUF; PE rate is
    identical to fp32r at 512-wide matmuls).  Accumulation stays fp32 in
    PSUM, so the only precision loss is input rounding (~0.4%), well within
    the 2e-2 gate.
  * Scores for the two heads of an m-chunk (contraction = dk = 64) are
    issued back-to-back as PE row-tiled matmuls (tile rows 0-63 / 64-127),
    which the PE streams concurrently.
  * The attention exp on the ACT engine is the hard wall (~(N+352)/1.2 ns
    per instr, dtype-independent).  The kernel is pipelined so ACT starts
    as soon as kproj+qproj(m0) are done and never starves: one exp instr
    covers both heads of a pair ([128, 1024] PSUM -> bf16 SBUF).
  * Key-padding mask applied by host-side COMPACTION of K/V (masked keys
    removed); `valid` zeroes the padded tail rows of V_aug so they add
    nothing to context or denominator (their exp(0)=1 is multiplied by 0).
  * Softmax denominator = 65th "ones" column of V_aug; normalization is
    fused into the PSUM evacuation (one tensor_tensor per head).
  * The out-projection for q-block i is interleaved into q-block i+1's
    attention so the PE never idles >3us (avoids HAM re-throttle to 1.2GHz).
"""
import numpy as np

B, S, D = 2, 2048, 1024
H, DK, DV = 16, 64, 64
SCALE = float(np.sqrt(DK))
NCORES = 8
GROUPS = 4           # head-groups (cores per batch)
HPG = H // GROUPS    # heads per core = 4
CH = HPG * DK        # channels per core = 256
MC = CH // 128       # m-chunks = head-pairs = 2
DJ = D // 128        # contraction chunks = 8
P = 128
QB = 512             # q-block width
NQB = S // QB        # 4

_BUILD_CACHE = {}
LAST_RESULTS = None  # test harness can read exec_time_ns etc. from here


def _bf16(a: np.ndarray):
    import ml_dtypes
    return np.ascontiguousarray(np.asarray(a, np.float32)).astype(ml_dtypes.bfloat16)


def _build(n_kp: int):
    """Build + schedule the per-core Bass program for a padded key count."""
    import concourse.bass as bass  # noqa: F401
    from concourse import bacc, tile, mybir

    DT = mybir.dt
    F32, BF16 = DT.float32, DT.bfloat16
    AF = mybir.ActivationFunctionType
    ALU = mybir.AluOpType

    NJ = n_kp // P                      # k-chunks
    NKB = (n_kp + 511) // 512           # 512-wide k blocks for the k projection

    nc = bacc.Bacc("TRN2", target_bir_lowering=False, debug=False,
                   num_devices=NCORES)

    xqT = nc.dram_tensor("xqT", [D, S], BF16, kind="ExternalInput")
    xkT = nc.dram_tensor("xkT", [D, n_kp], BF16, kind="ExternalInput")
    xvT = nc.dram_tensor("xvT", [D, n_kp], BF16, kind="ExternalInput")
    wqT = nc.dram_tensor("wqT", [D, CH], BF16, kind="ExternalInput")
    wkT = nc.dram_tensor("wkT", [D, CH], BF16, kind="ExternalInput")
    wvT = nc.dram_tensor("wvT", [D, CH], BF16, kind="ExternalInput")
    woT = nc.dram_tensor("woT", [CH, D], BF16, kind="ExternalInput")
    bq = nc.dram_tensor("bq", [CH], F32, kind="ExternalInput")
    bk = nc.dram_tensor("bk", [CH], F32, kind="ExternalInput")
    bv = nc.dram_tensor("bv", [CH], F32, kind="ExternalInput")
    valid = nc.dram_tensor("valid", [n_kp], F32, kind="ExternalInput")
    out = nc.dram_tensor("out", [S, D], BF16, kind="ExternalOutput")

    with tile.TileContext(nc) as tc:
        with (
            tc.tile_pool(name="xs", bufs=10) as xs,
            tc.tile_pool(name="persist", bufs=1) as pp,
            tc.tile_pool(name="exps", bufs=4) as ep,
            tc.tile_pool(name="scratch", bufs=4) as scr,
            tc.tile_pool(name="outs", bufs=3) as op,
            tc.tile_pool(name="smalls", bufs=4) as smalls,
            tc.tile_pool(name="psA", bufs=2, space="PSUM") as psA,
            tc.tile_pool(name="psB", bufs=1, space="PSUM") as psB,
            tc.tile_pool(name="psC", bufs=2, space="PSUM") as psC,
            tc.tile_pool(name="dscr", bufs=3, space="DRAM") as dscr,
        ):
            # ---- persistent SBUF ------------------------------------------
            wq_sb = pp.tile([P, DJ, CH], BF16, name="wq_sb")
            wk_sb = pp.tile([P, DJ, CH], BF16, name="wk_sb")
            wv_sb = pp.tile([P, DJ, CH], BF16, name="wv_sb")
            wo_sb = pp.tile([P, MC, D], BF16, name="wo_sb")
            bq_sb = pp.tile([P, MC], F32, name="bq_sb")
            bk_sb = pp.tile([P, MC], F32, name="bk_sb")
            qT_sb = pp.tile([P, MC, S], BF16, name="qT_sb")
            kT_sb = pp.tile([P, MC, n_kp], BF16, name="kT_sb")
            vaug = pp.tile([P, NJ, HPG, DV + 1], BF16, name="vaug")
            ctxN = pp.tile([P, MC, S], BF16, name="ctxN")

            # ---- DMA preamble (issue order == consumption order) ----------
            nc.sync.dma_start(out=wk_sb[:, 0, :], in_=wkT.ap()[0:P, :])
            nc.sync.dma_start(out=bk_sb[:], in_=bk.ap().rearrange("(m p) -> p m", p=P))
            xk_t = [xs.tile([P, S], BF16, tag="x", name=f"xk{dj}") for dj in range(DJ)]
            nc.sync.dma_start(out=xk_t[0][:, :n_kp], in_=xkT.ap()[0:P, :])
            for dj in range(1, DJ):
                nc.sync.dma_start(out=wk_sb[:, dj, :], in_=wkT.ap()[dj * P:(dj + 1) * P, :])
                nc.sync.dma_start(out=xk_t[dj][:, :n_kp], in_=xkT.ap()[dj * P:(dj + 1) * P, :])
            # v inputs next (vaug is needed by the first AV, right after the
            # first exp), then q inputs, then wo.
            xv_t = [xs.tile([P, S], BF16, tag="x", name=f"xv{dj}") for dj in range(DJ)]
            for dj in range(DJ):
                nc.sync.dma_start(out=wv_sb[:, dj, :], in_=wvT.ap()[dj * P:(dj + 1) * P, :])
                nc.sync.dma_start(out=xv_t[dj][:, :n_kp], in_=xvT.ap()[dj * P:(dj + 1) * P, :])
            bv_rep = pp.tile([P, CH], F32, name="bv_rep")
            nc.gpsimd.dma_start(out=bv_rep[:], in_=bv.ap()[None, :].partition_broadcast(P))
            valid_sb = pp.tile([P, NJ], F32, name="valid_sb")
            nc.sync.dma_start(out=valid_sb[:], in_=valid.ap().rearrange("(j p) -> p j", p=P))
            valid_bf = pp.tile([P, NJ], BF16, name="valid_bf")
            nc.vector.tensor_copy(out=valid_bf[:], in_=valid_sb[:])
            xq_t = [xs.tile([P, S], BF16, tag="x", name=f"xq{dj}") for dj in range(DJ)]
            nc.sync.dma_start(out=bq_sb[:], in_=bq.ap().rearrange("(m p) -> p m", p=P))
            for dj in range(DJ):
                nc.sync.dma_start(out=wq_sb[:, dj, :], in_=wqT.ap()[dj * P:(dj + 1) * P, :])
                nc.sync.dma_start(out=xq_t[dj][:], in_=xqT.ap()[dj * P:(dj + 1) * P, :])
            for m2 in range(MC):
                nc.sync.dma_start(out=wo_sb[:, m2, :], in_=woT.ap()[m2 * P:(m2 + 1) * P, :])

            # ---- k projection: kT[c, s] = sum_d WkT[d,c] * XkT[d,s] (+bk) --
            def emit_kproj(m):
                for kb in range(NKB):
                    w = min(512, n_kp - kb * 512)
                    ps = psA.tile([P, 1024], F32, tag="ps")
                    for dj in range(DJ):
                        nc.tensor.matmul(
                            ps[:, :w],
                            lhsT=wk_sb[:, dj, m * P:(m + 1) * P],
                            rhs=xk_t[dj][:, kb * 512:kb * 512 + w],
                            start=(dj == 0), stop=(dj == DJ - 1))
                    nc.vector.tensor_scalar(
                        out=kT_sb[:, m, kb * 512:kb * 512 + w], in0=ps[:, :w],
                        scalar1=bk_sb[:, m:m + 1], scalar2=None, op0=ALU.add)

            # ---- v projection: v[s, c] (+bv, *valid), build V_aug ---------
            def emit_vproj():
                for j in range(NJ):
                    ps = psA.tile([P, 1024], F32, tag="ps")
                    for dj in range(DJ):
                        nc.tensor.matmul(
                            ps[:, :CH],
                            lhsT=xv_t[dj][:, j * P:(j + 1) * P],
                            rhs=wv_sb[:, dj, :],
                            start=(dj == 0), stop=(dj == DJ - 1))
                    vst = scr.tile([P, 1024], F32, tag="s")
                    nc.vector.tensor_tensor(out=vst[:, :CH], in0=ps[:, :CH],
                                            in1=bv_rep[:], op=ALU.add)
                    nc.gpsimd.tensor_scalar(
                        out=vaug[:, j, :, 0:DV],
                        in0=vst[:, :CH].rearrange("p (h d) -> p h d", h=HPG),
                        scalar1=valid_sb[:, j:j + 1], scalar2=None, op0=ALU.mult)
                    for h in range(HPG):
                        nc.gpsimd.tensor_copy(out=vaug[:, j, h, DV:DV + 1],
                                              in_=valid_bf[:, j:j + 1])

            # ---- q projection (scale folded into Wq/bq) -------------------
            def emit_qproj(m):
                for qb in range(S // 512):
                    ps = psA.tile([P, 1024], F32, tag="ps")
                    for dj in range(DJ):
                        nc.tensor.matmul(
                            ps[:, :512],
                            lhsT=wq_sb[:, dj, m * P:(m + 1) * P],
                            rhs=xq_t[dj][:, qb * 512:(qb + 1) * 512],
                            start=(dj == 0), stop=(dj == DJ - 1))
                    nc.vector.tensor_scalar(
                        out=qT_sb[:, m, qb * 512:(qb + 1) * 512], in0=ps[:, :512],
                        scalar1=bq_sb[:, m:m + 1], scalar2=None, op0=ALU.add)

            # ---- attention for one (pair p, q-block qb) -------------------
            # ST: both heads of the pair issued back-to-back as row-tiled
            # K=64 matmuls (PE rows 0-63 / 64-127) -> stream concurrently.
            # One exp instr covers both heads' scores [128, 1024].
            # AV accumulates ctx^T (+denominator row 64) per head into one
            # shared [128, 1024] PSUM tile (head0 cols 0-511, head1 512-1023).
            def emit_attention(p, qb):
                q0 = qb * QB
                ctx_ps = psB.tile([P, 1024], F32, tag="ctx", name=f"ctx{p}{qb}")
                pending = None

                def emit_av(j, ex):
                    for hh in range(2):
                        nc.tensor.matmul(
                            ctx_ps[0:DV + 1, hh * 512:hh * 512 + 512],
                            lhsT=vaug[:, j, 2 * p + hh, :],
                            rhs=ex[:, hh * 512:(hh + 1) * 512],
                            start=(j == 0), stop=(j == NJ - 1),
                            skip_group_check=True)

                for j in range(NJ):
                    st = psA.tile([P, 1024], F32, tag="ps", name=f"st{p}{qb}{j}")
                    for hh in range(2):
                        po = hh * 64
                        nc.tensor.matmul(
                            st[:, hh * 512:(hh + 1) * 512],
                            lhsT=kT_sb[po:po + 64, p, j * P:(j + 1) * P],
                            rhs=qT_sb[po:po + 64, p, q0:q0 + 512],
                            start=True, stop=True)
                    ex = ep.tile([P, 1024], BF16, tag="e", name=f"ex{p}{qb}{j}")
                    nc.scalar.activation(out=ex[:], in_=st[:], func=AF.Exp)
                    if pending is not None:
                        emit_av(*pending)
                    pending = (j, ex)
                emit_av(*pending)
                return ctx_ps

            # ---- normalize pair (p, qb): fused evac + 1/denominator ------
            def emit_normalize(p, qb, ctx_ps):
                q0 = qb * QB
                # DVE can't shift partitions: the denominator row (PSUM
                # partition 64) is copied to SBUF partition 64, then DMA'd.
                den = scr.tile([P, 1024], F32, tag="s", name=f"dn{p}{qb}")
                nc.vector.tensor_copy(out=den[DV:DV + 1, :], in_=ctx_ps[DV:DV + 1, :])
                rb = dscr.tile([1, 1024], F32, tag="rb")
                nc.sync.dma_start(out=rb[:], in_=den[DV:DV + 1, :])
                rsq = smalls.tile([P, 8], F32, tag="rsq")
                nc.sync.dma_start(out=rsq[:], in_=rb.rearrange("o (p a) -> (o p) a", p=P))
                rcq = smalls.tile([P, 8], F32, tag="rcq")
                nc.vector.reciprocal(out=rcq[:], in_=rsq[:])
                rb2 = dscr.tile([1, 1024], F32, tag="rb2")
                nc.sync.dma_start(out=rb2.rearrange("o (p a) -> (o p) a", p=P), in_=rcq[:])
                recb = scr.tile([P, 1024], F32, tag="s", name=f"rc{p}{qb}")
                nc.gpsimd.dma_start(out=recb[0:64, :],
                                    in_=rb2[0][None, :].partition_broadcast(64))
                # head 2p (even, target partitions 0-63): straight in.
                nc.vector.tensor_tensor(
                    out=ctxN[0:64, p, q0:q0 + QB],
                    in0=ctx_ps[0:64, 0:512], in1=recb[0:64, 0:512], op=ALU.mult)
                # head 2p+1 (odd, target partitions 64-127): DVE can't shift
                # partitions; bounce through a small SB->SB DMA.
                tmp = scr.tile([P, 1024], BF16, tag="s", name=f"tm{p}{qb}")
                nc.vector.tensor_tensor(
                    out=tmp[0:64, 0:512],
                    in0=ctx_ps[0:64, 512:1024], in1=recb[0:64, 512:1024], op=ALU.mult)
                nc.sync.dma_start(out=ctxN[64:128, p, q0:q0 + QB], in_=tmp[0:64, 0:512])

            # ---- out-projection for one 128-row q chunk -------------------
            def emit_outproj(qc):
                stage = op.tile([P, D], BF16, tag="o", name=f"og{qc}")
                for n2 in range(2):
                    ps = psC.tile([P, 512], F32, tag="op")
                    for m in range(MC):
                        nc.tensor.matmul(
                            ps[:],
                            lhsT=ctxN[:, m, qc * P:(qc + 1) * P],
                            rhs=wo_sb[:, m, n2 * 512:(n2 + 1) * 512],
                            start=(m == 0), stop=(m == MC - 1))
                    nc.vector.tensor_copy(out=stage[:, n2 * 512:(n2 + 1) * 512], in_=ps[:])
                nc.sync.dma_start(out=out.ap()[qc * P:(qc + 1) * P, :], in_=stage[:])

            # ---- schedule -------------------------------------------------
            emit_kproj(0)
            emit_kproj(1)
            emit_vproj()
            emit_qproj(0)

            # qb0 attention for pair 0 starts the ACT stream; qproj(m1) fills
            # the PE while ACT chews on it.
            ctx00 = emit_attention(0, 0)
            emit_qproj(1)
            emit_normalize(0, 0, ctx00)
            ctx10 = emit_attention(1, 0)
            emit_normalize(1, 0, ctx10)
            prev_qb = 0
            for qb in range(1, NQB):
                ctxA = emit_attention(0, qb)
                emit_normalize(0, qb, ctxA)
                # out-projection of the previous q-block (both pairs done)
                for qc in range(prev_qb * 4, prev_qb * 4 + 2):
                    emit_outproj(qc)
                ctxB = emit_attention(1, qb)
                for qc in range(prev_qb * 4 + 2, prev_qb * 4 + 4):
                    emit_outproj(qc)
                emit_normalize(1, qb, ctxB)
                prev_qb = qb
            for qc in range(prev_qb * 4, prev_qb * 4 + 4):
                emit_outproj(qc)

    nc.compile()
    return nc


def _ensure_axon_hooks():
    """bass_utils imports antenv.axon_hooks when tracing; this image's antenv
    lacks it. Provide it, backed by the ctypes NTFF hook when available."""
    import sys
    import types
    try:
        import antenv.axon_hooks  # noqa: F401
        return
    except ImportError:
        pass
    hook = None
    try:
        from trn_agent_boot.trn_boot import _ntff_profile_via_ctypes
        hook = _ntff_profile_via_ctypes("/opt/axon/libaxon_pjrt.so")
    except Exception:
        hook = None
    mod = types.ModuleType("antenv.axon_hooks")
    mod._hook = hook
    mod.get_axon_ntff_profile_hook = lambda: mod._hook
    mod.set_axon_ntff_profile_hook = lambda h: setattr(mod, "_hook", h)
    sys.modules["antenv.axon_hooks"] = mod


def kernel(Q, K, V, atte_mask_out, Wq, bq, Wk, bk, Wv, bv, Wo, bo):
    import jax  # noqa: F401  (must be imported first so the axon backend registers)
    from concourse.bass_utils import run_bass_kernel_spmd
    global LAST_RESULTS
    _ensure_axon_hooks()

    Q = np.asarray(Q); K = np.asarray(K); V = np.asarray(V)
    mask = np.asarray(atte_mask_out).reshape(B, S)
    Wq = np.asarray(Wq); Wk = np.asarray(Wk); Wv = np.asarray(Wv); Wo = np.asarray(Wo)
    bq = np.asarray(bq); bk = np.asarray(bk); bv = np.asarray(bv); bo = np.asarray(bo)

    keep = [np.flatnonzero(~mask[b]) for b in range(B)]
    n_kp = max(P, max(((len(ix) + P - 1) // P) * P for ix in keep))

    # per-batch packed bf16 tensors
    xqT, xkT, xvT, validv = [], [], [], []
    for b in range(B):
        ix = keep[b]
        xqT.append(_bf16(Q[b].T))
        kk = np.zeros((D, n_kp), np.float32)
        vv = np.zeros((D, n_kp), np.float32)
        kk[:, :len(ix)] = K[b][ix].T
        vv[:, :len(ix)] = V[b][ix].T
        xkT.append(_bf16(kk))
        xvT.append(_bf16(vv))
        va = np.zeros(n_kp, np.float32)
        va[:len(ix)] = 1.0
        validv.append(va)

    in_maps = []
    for c in range(NCORES):
        b, g = c // GROUPS, c % GROUPS
        sl = slice(g * CH, (g + 1) * CH)
        in_maps.append({
            "xqT": xqT[b], "xkT": xkT[b], "xvT": xvT[b],
            "wqT": _bf16(Wq[sl].T / SCALE),
            "wkT": _bf16(Wk[sl].T),
            "wvT": _bf16(Wv[sl].T),
            "woT": _bf16(Wo[:, sl].T),
            "bq": np.ascontiguousarray(bq[sl] / SCALE, np.float32),
            "bk": np.ascontiguousarray(bk[sl], np.float32),
            "bv": np.ascontiguousarray(bv[sl], np.float32),
            "valid": validv[b],
        })

    if n_kp not in _BUILD_CACHE:
        _BUILD_CACHE[n_kp] = _build(n_kp)
    nc = _BUILD_CACHE[n_kp]

    res = run_bass_kernel_spmd(nc, in_maps, core_ids=list(range(NCORES)))
    LAST_RESULTS = res

    full = np.zeros((B, S, D), np.float32)
    full += bo.astype(np.float32)
    for c in range(NCORES):
        full[c // GROUPS] += np.asarray(res.results[c]["out"], np.float32)
    return full


# revision 10
# speedup vs baseline: 1.4698x; 1.4698x over previous
"""Multi-head attention (B=2, S=2048, D=1024, H=16, dk=dv=64) on 8 TRN2 NeuronCores.

Sharding: core c -> (batch b = c//4, head-group g = c%4, 4 heads each).
Each core computes q/k/v projections for its 4 heads (weight-column shard),
attention over its batch, and a partial output projection over its 256
channels (weight-row shard of Wo).  The host sums the 4 partial outputs per
batch at unshard time (the "all-reduce after the output projection").

v3 design: the ACT engine's exp stream is the hard lower bound
(64 x (1024+352)/1.2 ns ~= 73us, dtype-independent), so the whole kernel is
scheduled around keeping ACT saturated from ~20us on:

  * All matmul operands are bf16 (halves DMA; PE rate = fp32r at 512-wide).
  * Scores for the two heads of an m-chunk (K = dk = 64) are issued
    back-to-back as PE row-tiled matmuls (rows 0-63 / 64-127) -> they
    stream concurrently; one exp instr covers both heads [128, 1024].
  * Global software pipeline: for each score group g = (pair, qb, j):
    emit ST(g); exp(g); then <=2 "filler" PE pieces (deferred qproj/kproj/
    vproj/out-proj matmuls, 512-row granularity) from a queue; then AV(g-1).
    The PE never runs a multi-us block that would starve ACT, and never
    idles >3us (which would HAM-throttle it to 1.2 GHz).
  * Attention context is evacuated from PSUM to SBUF immediately after the
    last AV of a (pair, qb) so the single ctx PSUM buffer recycles fast;
    the softmax normalization (1/denominator from the 65th "ones" column
    of V_aug) happens from SBUF off the critical path.
  * Key-padding mask applied by host-side COMPACTION of K/V; `valid`
    zeroes padded tail rows of V_aug (their exp(0)=1 x 0 adds nothing).
"""
import numpy as np

B, S, D = 2, 2048, 1024
H, DK, DV = 16, 64, 64
SCALE = float(np.sqrt(DK))
NCORES = 8
GROUPS = 4           # head-groups (cores per batch)
HPG = H // GROUPS    # heads per core = 4
CH = HPG * DK        # channels per core = 256
MC = CH // 128       # m-chunks = head-pairs = 2
DJ = D // 128        # contraction chunks = 8
P = 128
QB = 512             # q-block width
NQB = S // QB        # 4

_BUILD_CACHE = {}
LAST_RESULTS = None  # test harness can read exec_time_ns etc. from here


def _bf16(a: np.ndarray):
    import ml_dtypes
    return np.ascontiguousarray(np.asarray(a, np.float32)).astype(ml_dtypes.bfloat16)


def _build(n_kp: int):
    """Build + schedule the per-core Bass program for a padded key count."""
    import concourse.bass as bass  # noqa: F401
    from concourse import bacc, tile, mybir
    from collections import deque

    DT = mybir.dt
    F32, BF16 = DT.float32, DT.bfloat16
    AF = mybir.ActivationFunctionType
    ALU = mybir.AluOpType

    NJ = n_kp // P                      # k-chunks
    NKB = (n_kp + 511) // 512           # 512-wide k blocks for the k projection

    nc = bacc.Bacc("TRN2", target_bir_lowering=False, debug=False,
                   num_devices=NCORES)

    xqT = nc.dram_tensor("xqT", [D, S], BF16, kind="ExternalInput")
    xkT = nc.dram_tensor("xkT", [D, n_kp], BF16, kind="ExternalInput")
    xvT = nc.dram_tensor("xvT", [D, n_kp], BF16, kind="ExternalInput")
    wqT = nc.dram_tensor("wqT", [D, CH], BF16, kind="ExternalInput")
    wkT = nc.dram_tensor("wkT", [D, CH], BF16, kind="ExternalInput")
    wvT = nc.dram_tensor("wvT", [D, CH], BF16, kind="ExternalInput")
    woT = nc.dram_tensor("woT", [CH, D], BF16, kind="ExternalInput")
    bq = nc.dram_tensor("bq", [CH], F32, kind="ExternalInput")
    bk = nc.dram_tensor("bk", [CH], F32, kind="ExternalInput")
    bv = nc.dram_tensor("bv", [CH], F32, kind="ExternalInput")
    valid = nc.dram_tensor("valid", [n_kp], F32, kind="ExternalInput")
    out = nc.dram_tensor("out", [S, D], BF16, kind="ExternalOutput")

    with tile.TileContext(nc) as tc:
        with (
            tc.tile_pool(name="xs", bufs=3 * DJ) as xs,
            tc.tile_pool(name="persist", bufs=1) as pp,
            tc.tile_pool(name="exps", bufs=4) as ep,
            tc.tile_pool(name="scratch", bufs=4) as scr,
            tc.tile_pool(name="cu", bufs=2) as cu,
            tc.tile_pool(name="outs", bufs=3) as op,
            tc.tile_pool(name="smalls", bufs=4) as smalls,
            tc.tile_pool(name="psA", bufs=2, space="PSUM") as psA,
            tc.tile_pool(name="psB", bufs=1, space="PSUM") as psB,
            tc.tile_pool(name="psC", bufs=2, space="PSUM") as psC,
            tc.tile_pool(name="dscr", bufs=3, space="DRAM") as dscr,
        ):
            # ---- persistent SBUF ------------------------------------------
            wq_sb = pp.tile([P, DJ, CH], BF16, name="wq_sb")
            wk_sb = pp.tile([P, DJ, CH], BF16, name="wk_sb")
            wv_sb = pp.tile([P, DJ, CH], BF16, name="wv_sb")
            wo_sb = pp.tile([P, MC, D], BF16, name="wo_sb")
            bq_sb = pp.tile([P, MC], F32, name="bq_sb")
            bk_sb = pp.tile([P, MC], F32, name="bk_sb")
            qT_sb = pp.tile([P, MC, S], BF16, name="qT_sb")
            kT_sb = pp.tile([P, MC, n_kp], BF16, name="kT_sb")
            vaug = pp.tile([P, NJ, HPG, DV + 1], BF16, name="vaug")
            ctxN = pp.tile([P, MC, S], BF16, name="ctxN")
            bv_rep = pp.tile([P, CH], F32, name="bv_rep")
            valid_sb = pp.tile([P, NJ], F32, name="valid_sb")
            valid_bf = pp.tile([P, NJ], BF16, name="valid_bf")

            # ---- DMA preamble, split across engine queues -----------------
            # sync queue: k inputs (first PE work), then q inputs (qb0
            # columns first so attention can start), then the q remainder.
            xk_t = [xs.tile([P, S], BF16, tag="x", name=f"xk{dj}") for dj in range(DJ)]
            xv_t = [xs.tile([P, S], BF16, tag="x", name=f"xv{dj}") for dj in range(DJ)]
            xq_t = [xs.tile([P, S], BF16, tag="x", name=f"xq{dj}") for dj in range(DJ)]
            for dj in range(DJ):
                nc.sync.dma_start(out=xk_t[dj][:, :n_kp], in_=xkT.ap()[dj * P:(dj + 1) * P, :])
            for dj in range(DJ):
                nc.sync.dma_start(out=xq_t[dj][:, 0:QB], in_=xqT.ap()[dj * P:(dj + 1) * P, 0:QB])
            for dj in range(DJ):
                nc.sync.dma_start(out=xq_t[dj][:, QB:], in_=xqT.ap()[dj * P:(dj + 1) * P, QB:])
            # scalar queue (ACT idle until attention): weights + v inputs
            nc.scalar.dma_start(out=wk_sb[:], in_=wkT.ap().rearrange("(j p) c -> p j c", p=P))
            nc.scalar.dma_start(out=wv_sb[:], in_=wvT.ap().rearrange("(j p) c -> p j c", p=P))
            nc.scalar.dma_start(out=wq_sb[:], in_=wqT.ap().rearrange("(j p) c -> p j c", p=P))
            for dj in range(DJ):
                nc.scalar.dma_start(out=xv_t[dj][:, :n_kp], in_=xvT.ap()[dj * P:(dj + 1) * P, :])
            nc.scalar.dma_start(out=wo_sb[:], in_=woT.ap().rearrange("(m p) d -> p m d", p=P))
            # gpsimd queue: small constants
            nc.gpsimd.dma_start(out=bk_sb[:], in_=bk.ap().rearrange("(m p) -> p m", p=P))
            nc.gpsimd.dma_start(out=bq_sb[:], in_=bq.ap().rearrange("(m p) -> p m", p=P))
            nc.gpsimd.dma_start(out=bv_rep[:], in_=bv.ap()[None, :].partition_broadcast(P))
            nc.gpsimd.dma_start(out=valid_sb[:], in_=valid.ap().rearrange("(j p) -> p j", p=P))
            nc.vector.tensor_copy(out=valid_bf[:], in_=valid_sb[:])

            # ---- filler queue machinery -----------------------------------
            fillers = deque()   # (group, closure) - ~0.5us of PE work each
            _uid = [0]

            def uname(pfx):
                _uid[0] += 1
                return f"{pfx}{_uid[0]}"

            def drain(n):
                for _ in range(min(n, len(fillers))):
                    g, fn = fillers.popleft()
                    fn()

            def drain_groups(groups):
                """Emit every queued filler belonging to `groups` (and
                anything queued ahead of them - FIFO order preserved)."""
                while any(g in groups for g, _ in fillers):
                    g, fn = fillers.popleft()
                    fn()

            def drain_all():
                while fillers:
                    fillers.popleft()[1]()

            # ---- k projection ---------------------------------------------
            def kproj_mms(m, kb, dj0, dj1, st):
                if "ps" not in st:
                    st["ps"] = psC.tile([P, 512], F32, tag="pj", name=uname("kps"))
                w = min(512, n_kp - kb * 512)
                for dj in range(dj0, dj1):
                    nc.tensor.matmul(
                        st["ps"][:, :w],
                        lhsT=wk_sb[:, dj, m * P:(m + 1) * P],
                        rhs=xk_t[dj][:, kb * 512:kb * 512 + w],
                        start=(dj == 0), stop=(dj == DJ - 1),
                        skip_group_check=True)
                if dj1 == DJ:
                    nc.vector.tensor_scalar(
                        out=kT_sb[:, m, kb * 512:kb * 512 + w], in0=st["ps"][:, :w],
                        scalar1=bk_sb[:, m:m + 1], scalar2=None, op0=ALU.add)

            def emit_kproj(m):
                for kb in range(NKB):
                    kproj_mms(m, kb, 0, DJ, {})

            def push_kproj_fillers(m):
                for kb in range(NKB):
                    st = {}
                    for q in range(4):
                        fillers.append((f"kp{m}", (lambda kb=kb, q=q, st=st:
                                                   kproj_mms(m, kb, 2 * q, 2 * q + 2, st))))

            # ---- v projection ---------------------------------------------
            def vproj_mms(j, dj0, dj1, st):
                if "ps" not in st:
                    st["ps"] = psC.tile([P, 512], F32, tag="pj", name=uname("vps"))
                ps = st["ps"]
                for dj in range(dj0, dj1):
                    nc.tensor.matmul(
                        ps[:, :CH],
                        lhsT=xv_t[dj][:, j * P:(j + 1) * P],
                        rhs=wv_sb[:, dj, :],
                        start=(dj == 0), stop=(dj == DJ - 1),
                        skip_group_check=True)
                if dj1 == DJ:
                    vst = scr.tile([P, 1024], F32, tag="s", name=uname("vst"))
                    nc.vector.tensor_tensor(out=vst[:, :CH], in0=ps[:, :CH],
                                            in1=bv_rep[:], op=ALU.add)
                    nc.vector.tensor_scalar(
                        out=vaug[:, j, :, 0:DV],
                        in0=vst[:, :CH].rearrange("p (h d) -> p h d", h=HPG),
                        scalar1=valid_sb[:, j:j + 1], scalar2=None, op0=ALU.mult)
                    for h in range(HPG):
                        nc.gpsimd.tensor_copy(out=vaug[:, j, h, DV:DV + 1],
                                              in_=valid_bf[:, j:j + 1])

            def push_vproj_fillers(j):
                st = {}
                fillers.append((f"vp{j}", lambda j=j, st=st: vproj_mms(j, 0, 4, st)))
                fillers.append((f"vp{j}", lambda j=j, st=st: vproj_mms(j, 4, DJ, st)))

            # ---- q projection ---------------------------------------------
            def qproj_mms(m, qb, dj0, dj1, st):
                if "ps" not in st:
                    st["ps"] = psC.tile([P, 512], F32, tag="pj", name=uname("qps"))
                for dj in range(dj0, dj1):
                    nc.tensor.matmul(
                        st["ps"][:],
                        lhsT=wq_sb[:, dj, m * P:(m + 1) * P],
                        rhs=xq_t[dj][:, qb * 512:(qb + 1) * 512],
                        start=(dj == 0), stop=(dj == DJ - 1),
                        skip_group_check=True)
                if dj1 == DJ:
                    nc.vector.tensor_scalar(
                        out=qT_sb[:, m, qb * 512:(qb + 1) * 512], in0=st["ps"][:],
                        scalar1=bq_sb[:, m:m + 1], scalar2=None, op0=ALU.add)

            def push_qproj_fillers(m, qb):
                st = {}
                for q in range(4):
                    fillers.append((f"qp{m}{qb}", (lambda m=m, qb=qb, q=q, st=st:
                                                   qproj_mms(m, qb, 2 * q, 2 * q + 2, st))))

            # ---- out-projection (as fillers), per (q-chunk, n-half) -------
            op_stage = {}

            def outproj_piece(qc, n2):
                if qc not in op_stage:
                    op_stage[qc] = op.tile([P, D], BF16, tag="o", name=f"og{qc}")
                stage = op_stage[qc]
                ps = psC.tile([P, 512], F32, tag="pj", name=uname("ops"))
                for m in range(MC):
                    nc.tensor.matmul(
                        ps[:],
                        lhsT=ctxN[:, m, qc * P:(qc + 1) * P],
                        rhs=wo_sb[:, m, n2 * 512:(n2 + 1) * 512],
                        start=(m == 0), stop=(m == MC - 1),
                        skip_group_check=True)
                nc.vector.tensor_copy(out=stage[:, n2 * 512:(n2 + 1) * 512], in_=ps[:])
                if n2 == 1:
                    nc.sync.dma_start(out=out.ap()[qc * P:(qc + 1) * P, :], in_=stage[:])
                    del op_stage[qc]

            def push_outproj_fillers(qb):
                for qc in range(qb * 4, qb * 4 + 4):
                    for n2 in range(2):
                        fillers.append(("op", lambda qc=qc, n2=n2: outproj_piece(qc, n2)))

            # ---- attention ------------------------------------------------
            # Pending AV is global so the one-group software skew spans pair
            # boundaries with no ACT gap at the seams.
            pend = {"av": None}

            def emit_av(p, qb, j, ex, ctx_ps, last):
                drain_groups({f"vp{j}"})  # vaug[j] producers must precede
                for hh in range(2):
                    nc.tensor.matmul(
                        ctx_ps[0:DV + 1, hh * 512:hh * 512 + 512],
                        lhsT=vaug[:, j, 2 * p + hh, :],
                        rhs=ex[:, hh * 512:(hh + 1) * 512],
                        start=(j == 0), stop=(j == NJ - 1),
                        skip_group_check=True)
                if last:
                    finish_pair(p, qb, ctx_ps)

            def flush_av():
                if pend["av"] is not None:
                    fn = pend["av"]
                    pend["av"] = None
                    fn()

            def emit_attention(p, qb):
                # safety: inputs of this block must already be emitted
                drain_groups({f"kp{p}", f"qp{p}{qb}"})
                q0 = qb * QB
                ctx_ps = psB.tile([P, 1024], F32, tag="ctx", name=f"ctx{p}{qb}")
                for j in range(NJ):
                    st = psA.tile([P, 1024], F32, tag="st", name=f"st{p}{qb}{j}")
                    for hh in range(2):
                        po = hh * 64
                        nc.tensor.matmul(
                            st[:, hh * 512:(hh + 1) * 512],
                            lhsT=kT_sb[po:po + 64, p, j * P:(j + 1) * P],
                            rhs=qT_sb[po:po + 64, p, q0:q0 + 512],
                            start=True, stop=True)
                    ex = ep.tile([P, 1024], BF16, tag="e", name=f"ex{p}{qb}{j}")
                    nc.scalar.activation(out=ex[:], in_=st[:], func=AF.Exp)
                    k = 2 if len(fillers) > 10 else 1
                    if pend["av"] is not None and getattr(pend["av"], "last", False):
                        # release the ctx PSUM buffer ASAP at pair seams
                        flush_av()
                        drain(k)
                    else:
                        drain(k)
                        flush_av()
                    fn = (lambda p=p, qb=qb, j=j, ex=ex, ctx_ps=ctx_ps,
                          last=(j == NJ - 1): emit_av(p, qb, j, ex, ctx_ps, last))
                    fn.last = (j == NJ - 1)
                    pend["av"] = fn

            # ---- finish a (pair, qb): evacuate ctx, normalize -------------
            def finish_pair(p, qb, ctx_ps):
                q0 = qb * QB
                ctxU = cu.tile([P, 1024], F32, tag="cu", name=f"cu{p}{qb}")
                nc.vector.tensor_copy(out=ctxU[0:DV + 1, :], in_=ctx_ps[0:DV + 1, :])
                # reciprocal of the 1024 denominators via a [128, 8] reshape
                # (single-partition DVE reciprocal is ~13us); DRAM bounces
                # do the reshape; all hops on the gpsimd DMA queue.
                rb = dscr.tile([1, 1024], F32, tag="rb")
                nc.gpsimd.dma_start(out=rb[:], in_=ctxU[DV:DV + 1, :])
                rsq = smalls.tile([P, 8], F32, tag="rsq")
                nc.gpsimd.dma_start(out=rsq[:], in_=rb.rearrange("o (p a) -> (o p) a", p=P))
                rcq = smalls.tile([P, 8], F32, tag="rcq")
                nc.vector.reciprocal(out=rcq[:], in_=rsq[:])
                rb2 = dscr.tile([1, 1024], F32, tag="rb2")
                nc.gpsimd.dma_start(out=rb2.rearrange("o (p a) -> (o p) a", p=P), in_=rcq[:])
                recb = scr.tile([P, 1024], F32, tag="s", name=f"rc{p}{qb}")
                nc.gpsimd.dma_start(out=recb[0:64, :],
                                    in_=rb2[0][None, :].partition_broadcast(64))
                # head 2p (even -> partitions 0-63) straight into ctxN
                nc.vector.tensor_tensor(
                    out=ctxN[0:64, p, q0:q0 + QB],
                    in0=ctxU[0:64, 0:512], in1=recb[0:64, 0:512], op=ALU.mult)
                # head 2p+1 (odd -> partitions 64-127) via SB->SB DMA shift
                tmp = scr.tile([P, 1024], BF16, tag="s", name=f"tm{p}{qb}")
                nc.vector.tensor_tensor(
                    out=tmp[0:64, 0:512],
                    in0=ctxU[0:64, 512:1024], in1=recb[0:64, 512:1024], op=ALU.mult)
                nc.sync.dma_start(out=ctxN[64:128, p, q0:q0 + QB], in_=tmp[0:64, 0:512])
                if p == 1:
                    push_outproj_fillers(qb)

            # ---- schedule -------------------------------------------------
            emit_kproj(0)
            for j in range(min(5, NJ)):
                vproj_mms(j, 0, DJ, {})
            qproj_mms(0, 0, 0, DJ, {})

            for j in range(5, NJ):
                push_vproj_fillers(j)
            push_kproj_fillers(1)
            for qb in range(NQB):
                for m in range(MC):
                    if (m, qb) != (0, 0):
                        push_qproj_fillers(m, qb)

            for qb in range(NQB):
                emit_attention(0, qb)
                emit_attention(1, qb)
            flush_av()          # last AV + finish_pair(1, NQB-1)
            drain_all()         # out-projection of the last q-block

    nc.compile()
    return nc


def _ensure_axon_hooks():
    """bass_utils imports antenv.axon_hooks when tracing; this image's antenv
    lacks it. Provide it, backed by the ctypes NTFF hook when available."""
    import sys
    import types
    try:
        import antenv.axon_hooks  # noqa: F401
        return
    except ImportError:
        pass
    hook = None
    try:
        from trn_agent_boot.trn_boot import _ntff_profile_via_ctypes
        hook = _ntff_profile_via_ctypes("/opt/axon/libaxon_pjrt.so")
    except Exception:
        hook = None
    mod = types.ModuleType("antenv.axon_hooks")
    mod._hook = hook
    mod.get_axon_ntff_profile_hook = lambda: mod._hook
    mod.set_axon_ntff_profile_hook = lambda h: setattr(mod, "_hook", h)
    sys.modules["antenv.axon_hooks"] = mod


def kernel(Q, K, V, atte_mask_out, Wq, bq, Wk, bk, Wv, bv, Wo, bo):
    import jax  # noqa: F401  (must be imported first so the axon backend registers)
    from concourse.bass_utils import run_bass_kernel_spmd
    global LAST_RESULTS
    _ensure_axon_hooks()

    Q = np.asarray(Q); K = np.asarray(K); V = np.asarray(V)
    mask = np.asarray(atte_mask_out).reshape(B, S)
    Wq = np.asarray(Wq); Wk = np.asarray(Wk); Wv = np.asarray(Wv); Wo = np.asarray(Wo)
    bq = np.asarray(bq); bk = np.asarray(bk); bv = np.asarray(bv); bo = np.asarray(bo)

    keep = [np.flatnonzero(~mask[b]) for b in range(B)]
    n_kp = max(P, max(((len(ix) + P - 1) // P) * P for ix in keep))

    # per-batch packed bf16 tensors
    xqT, xkT, xvT, validv = [], [], [], []
    for b in range(B):
        ix = keep[b]
        xqT.append(_bf16(Q[b].T))
        kk = np.zeros((D, n_kp), np.float32)
        vv = np.zeros((D, n_kp), np.float32)
        kk[:, :len(ix)] = K[b][ix].T
        vv[:, :len(ix)] = V[b][ix].T
        xkT.append(_bf16(kk))
        xvT.append(_bf16(vv))
        va = np.zeros(n_kp, np.float32)
        va[:len(ix)] = 1.0
        validv.append(va)

    in_maps = []
    for c in range(NCORES):
        b, g = c // GROUPS, c % GROUPS
        sl = slice(g * CH, (g + 1) * CH)
        in_maps.append({
            "xqT": xqT[b], "xkT": xkT[b], "xvT": xvT[b],
            "wqT": _bf16(Wq[sl].T / SCALE),
            "wkT": _bf16(Wk[sl].T),
            "wvT": _bf16(Wv[sl].T),
            "woT": _bf16(Wo[:, sl].T),
            "bq": np.ascontiguousarray(bq[sl] / SCALE, np.float32),
            "bk": np.ascontiguousarray(bk[sl], np.float32),
            "bv": np.ascontiguousarray(bv[sl], np.float32),
            "valid": validv[b],
        })

    if n_kp not in _BUILD_CACHE:
        _BUILD_CACHE[n_kp] = _build(n_kp)
    nc = _BUILD_CACHE[n_kp]

    res = run_bass_kernel_spmd(nc, in_maps, core_ids=list(range(NCORES)))
    LAST_RESULTS = res

    full = np.zeros((B, S, D), np.float32)
    full += bo.astype(np.float32)
    for c in range(NCORES):
        full[c // GROUPS] += np.asarray(res.results[c]["out"], np.float32)
    return full


# revision 11
# speedup vs baseline: 1.5227x; 1.0360x over previous
"""Multi-head attention (B=2, S=2048, D=1024, H=16, dk=dv=64) on 8 TRN2 NeuronCores.

Sharding: core c -> (batch b = c//4, head-group g = c%4, 4 heads each).
Each core computes q/k/v projections for its 4 heads (weight-column shard),
attention over its batch, and a partial output projection over its 256
channels (weight-row shard of Wo).  The host sums the 4 partial outputs per
batch at unshard time (the "all-reduce after the output projection").

v3 design: the ACT engine's exp stream is the hard lower bound
(64 x (1024+352)/1.2 ns ~= 73us, dtype-independent), so the whole kernel is
scheduled around keeping ACT saturated from ~20us on:

  * All matmul operands are bf16 (halves DMA; PE rate = fp32r at 512-wide).
  * Scores for the two heads of an m-chunk (K = dk = 64) are issued
    back-to-back as PE row-tiled matmuls (rows 0-63 / 64-127) -> they
    stream concurrently; one exp instr covers both heads [128, 1024].
  * Global software pipeline: for each score group g = (pair, qb, j):
    emit ST(g); exp(g); then <=2 "filler" PE pieces (deferred qproj/kproj/
    vproj/out-proj matmuls, 512-row granularity) from a queue; then AV(g-1).
    The PE never runs a multi-us block that would starve ACT, and never
    idles >3us (which would HAM-throttle it to 1.2 GHz).
  * Attention context is evacuated from PSUM to SBUF immediately after the
    last AV of a (pair, qb) so the single ctx PSUM buffer recycles fast;
    the softmax normalization (1/denominator from the 65th "ones" column
    of V_aug) happens from SBUF off the critical path.
  * Key-padding mask applied by host-side COMPACTION of K/V; `valid`
    zeroes padded tail rows of V_aug (their exp(0)=1 x 0 adds nothing).
"""
import numpy as np

B, S, D = 2, 2048, 1024
H, DK, DV = 16, 64, 64
SCALE = float(np.sqrt(DK))
NCORES = 8
GROUPS = 4           # head-groups (cores per batch)
HPG = H // GROUPS    # heads per core = 4
CH = HPG * DK        # channels per core = 256
MC = CH // 128       # m-chunks = head-pairs = 2
DJ = D // 128        # contraction chunks = 8
P = 128
QB = 512             # q-block width
NQB = S // QB        # 4

_BUILD_CACHE = {}
LAST_RESULTS = None  # test harness can read exec_time_ns etc. from here


def _bf16(a: np.ndarray):
    import ml_dtypes
    return np.ascontiguousarray(np.asarray(a, np.float32)).astype(ml_dtypes.bfloat16)


def _wpack(wT: np.ndarray, cols: int) -> np.ndarray:
    """[J*128, cols] -> [128, J*cols]: row j*128+p lands at [p, j, :]."""
    J = wT.shape[0] // 128
    return np.ascontiguousarray(
        wT.reshape(J, 128, cols).transpose(1, 0, 2).reshape(128, J * cols))


def _build(n_kp: int):
    """Build + schedule the per-core Bass program for a padded key count."""
    import concourse.bass as bass  # noqa: F401
    from concourse import bacc, tile, mybir
    from collections import deque

    DT = mybir.dt
    F32, BF16 = DT.float32, DT.bfloat16
    AF = mybir.ActivationFunctionType
    ALU = mybir.AluOpType

    NJ = n_kp // P                      # k-chunks
    NKB = (n_kp + 511) // 512           # 512-wide k blocks for the k projection

    nc = bacc.Bacc("TRN2", target_bir_lowering=False, debug=False,
                   num_devices=NCORES)

    xkT = nc.dram_tensor("xkT", [D, n_kp], BF16, kind="ExternalInput")
    xvT = nc.dram_tensor("xvT", [D, n_kp], BF16, kind="ExternalInput")
    # weights arrive host-pre-shuffled so each is ONE contiguous DMA:
    # wxP[p, dj, c] = W.T[dj*128+p, c]; woP[p, m, d] = Wo.T[m*128+p, d]
    wqP = nc.dram_tensor("wqP", [P, DJ * CH], BF16, kind="ExternalInput")
    wkP = nc.dram_tensor("wkP", [P, DJ * CH], BF16, kind="ExternalInput")
    wvP = nc.dram_tensor("wvP", [P, DJ * CH], BF16, kind="ExternalInput")
    woP = nc.dram_tensor("woP", [P, MC * D], BF16, kind="ExternalInput")
    xq0 = nc.dram_tensor("xq0", [D, 512], BF16, kind="ExternalInput")
    xqR = nc.dram_tensor("xqR", [D, S - 512], BF16, kind="ExternalInput")
    bq = nc.dram_tensor("bq", [CH], F32, kind="ExternalInput")
    bk = nc.dram_tensor("bk", [CH], F32, kind="ExternalInput")
    bv = nc.dram_tensor("bv", [CH], F32, kind="ExternalInput")
    valid = nc.dram_tensor("valid", [n_kp], F32, kind="ExternalInput")
    out = nc.dram_tensor("out", [S, D], BF16, kind="ExternalOutput")

    with tile.TileContext(nc) as tc:
        with (
            tc.tile_pool(name="xs", bufs=3 * DJ) as xs,
            tc.tile_pool(name="persist", bufs=1) as pp,
            tc.tile_pool(name="exps", bufs=4) as ep,
            tc.tile_pool(name="scratch", bufs=4) as scr,
            tc.tile_pool(name="cu", bufs=2) as cu,
            tc.tile_pool(name="outs", bufs=3) as op,
            tc.tile_pool(name="smalls", bufs=4) as smalls,
            tc.tile_pool(name="psA", bufs=2, space="PSUM") as psA,
            tc.tile_pool(name="psB", bufs=1, space="PSUM") as psB,
            tc.tile_pool(name="psC", bufs=2, space="PSUM") as psC,
            tc.tile_pool(name="dscr", bufs=3, space="DRAM") as dscr,
        ):
            # ---- persistent SBUF ------------------------------------------
            wq_sb = pp.tile([P, DJ, CH], BF16, name="wq_sb")
            wk_sb = pp.tile([P, DJ, CH], BF16, name="wk_sb")
            wv_sb = pp.tile([P, DJ, CH], BF16, name="wv_sb")
            wo_sb = pp.tile([P, MC, D], BF16, name="wo_sb")
            bq_sb = pp.tile([P, MC], F32, name="bq_sb")
            bk_sb = pp.tile([P, MC], F32, name="bk_sb")
            qT_sb = pp.tile([P, MC, S], BF16, name="qT_sb")
            kT_sb = pp.tile([P, MC, n_kp], BF16, name="kT_sb")
            vaug = pp.tile([P, NJ, HPG, DV + 1], BF16, name="vaug")
            ctxN = pp.tile([P, MC, S], BF16, name="ctxN")
            bv_rep = pp.tile([P, CH], F32, name="bv_rep")
            valid_sb = pp.tile([P, NJ], F32, name="valid_sb")
            valid_bf = pp.tile([P, NJ], BF16, name="valid_bf")

            # ---- DMA preamble, split across engine queues -----------------
            # sync queue: k inputs (first PE work), then q inputs (qb0
            # columns first so attention can start), then the q remainder.
            xk_t = [xs.tile([P, S], BF16, tag="x", name=f"xk{dj}") for dj in range(DJ)]
            xv_t = [xs.tile([P, S], BF16, tag="x", name=f"xv{dj}") for dj in range(DJ)]
            xq_t = [xs.tile([P, S], BF16, tag="x", name=f"xq{dj}") for dj in range(DJ)]
            for dj in range(DJ):
                nc.sync.dma_start(out=xk_t[dj][:, :n_kp], in_=xkT.ap()[dj * P:(dj + 1) * P, :])
            for dj in range(DJ):
                nc.sync.dma_start(out=xq_t[dj][:, 0:QB], in_=xq0.ap()[dj * P:(dj + 1) * P, :])
            for dj in range(DJ):
                nc.sync.dma_start(out=xq_t[dj][:, QB:], in_=xqR.ap()[dj * P:(dj + 1) * P, :])
            # scalar queue (ACT idle until attention): weights + v inputs
            nc.scalar.dma_start(out=wk_sb.rearrange("p j c -> p (j c)"), in_=wkP.ap())
            nc.scalar.dma_start(out=wv_sb.rearrange("p j c -> p (j c)"), in_=wvP.ap())
            for dj in range(DJ):
                nc.scalar.dma_start(out=xv_t[dj][:, :n_kp], in_=xvT.ap()[dj * P:(dj + 1) * P, :])
            nc.scalar.dma_start(out=wq_sb.rearrange("p j c -> p (j c)"), in_=wqP.ap())
            nc.scalar.dma_start(out=wo_sb.rearrange("p m d -> p (m d)"), in_=woP.ap())
            # gpsimd queue: small constants
            nc.gpsimd.dma_start(out=bk_sb[:], in_=bk.ap().rearrange("(m p) -> p m", p=P))
            nc.gpsimd.dma_start(out=bq_sb[:], in_=bq.ap().rearrange("(m p) -> p m", p=P))
            nc.gpsimd.dma_start(out=bv_rep[:], in_=bv.ap()[None, :].partition_broadcast(P))
            nc.gpsimd.dma_start(out=valid_sb[:], in_=valid.ap().rearrange("(j p) -> p j", p=P))
            nc.vector.tensor_copy(out=valid_bf[:], in_=valid_sb[:])

            # ---- filler queue machinery -----------------------------------
            fillers = deque()   # (group, closure) - ~0.5us of PE work each
            _uid = [0]

            def uname(pfx):
                _uid[0] += 1
                return f"{pfx}{_uid[0]}"

            def drain(n):
                for _ in range(min(n, len(fillers))):
                    g, fn = fillers.popleft()
                    fn()

            def drain_groups(groups):
                """Emit every queued filler belonging to `groups` (and
                anything queued ahead of them - FIFO order preserved)."""
                while any(g in groups for g, _ in fillers):
                    g, fn = fillers.popleft()
                    fn()

            def drain_all():
                while fillers:
                    fillers.popleft()[1]()

            # ---- k projection ---------------------------------------------
            def kproj_mms(m, kb, dj0, dj1, st):
                if "ps" not in st:
                    st["ps"] = psC.tile([P, 512], F32, tag="pj", name=uname("kps"))
                w = min(512, n_kp - kb * 512)
                for dj in range(dj0, dj1):
                    nc.tensor.matmul(
                        st["ps"][:, :w],
                        lhsT=wk_sb[:, dj, m * P:(m + 1) * P],
                        rhs=xk_t[dj][:, kb * 512:kb * 512 + w],
                        start=(dj == 0), stop=(dj == DJ - 1),
                        skip_group_check=True)
                if dj1 == DJ:
                    nc.vector.tensor_scalar(
                        out=kT_sb[:, m, kb * 512:kb * 512 + w], in0=st["ps"][:, :w],
                        scalar1=bk_sb[:, m:m + 1], scalar2=None, op0=ALU.add)

            def emit_kproj(m):
                for kb in range(NKB):
                    kproj_mms(m, kb, 0, DJ, {})

            def push_kproj_fillers(m):
                for kb in range(NKB):
                    st = {}
                    for q in range(4):
                        fillers.append((f"kp{m}", (lambda kb=kb, q=q, st=st:
                                                   kproj_mms(m, kb, 2 * q, 2 * q + 2, st))))

            # ---- v projection ---------------------------------------------
            def vproj_mms(j, dj0, dj1, st):
                if "ps" not in st:
                    st["ps"] = psC.tile([P, 512], F32, tag="pj", name=uname("vps"))
                ps = st["ps"]
                for dj in range(dj0, dj1):
                    nc.tensor.matmul(
                        ps[:, :CH],
                        lhsT=xv_t[dj][:, j * P:(j + 1) * P],
                        rhs=wv_sb[:, dj, :],
                        start=(dj == 0), stop=(dj == DJ - 1),
                        skip_group_check=True)
                if dj1 == DJ:
                    vst = scr.tile([P, 1024], F32, tag="s", name=uname("vst"))
                    nc.vector.tensor_tensor(out=vst[:, :CH], in0=ps[:, :CH],
                                            in1=bv_rep[:], op=ALU.add)
                    nc.vector.tensor_scalar(
                        out=vaug[:, j, :, 0:DV],
                        in0=vst[:, :CH].rearrange("p (h d) -> p h d", h=HPG),
                        scalar1=valid_sb[:, j:j + 1], scalar2=None, op0=ALU.mult)
                    for h in range(HPG):
                        nc.gpsimd.tensor_copy(out=vaug[:, j, h, DV:DV + 1],
                                              in_=valid_bf[:, j:j + 1])

            def push_vproj_fillers(j):
                st = {}
                fillers.append((f"vp{j}", lambda j=j, st=st: vproj_mms(j, 0, 4, st)))
                fillers.append((f"vp{j}", lambda j=j, st=st: vproj_mms(j, 4, DJ, st)))

            # ---- q projection ---------------------------------------------
            def qproj_mms(m, qb, dj0, dj1, st):
                if "ps" not in st:
                    st["ps"] = psC.tile([P, 512], F32, tag="pj", name=uname("qps"))
                for dj in range(dj0, dj1):
                    nc.tensor.matmul(
                        st["ps"][:],
                        lhsT=wq_sb[:, dj, m * P:(m + 1) * P],
                        rhs=xq_t[dj][:, qb * 512:(qb + 1) * 512],
                        start=(dj == 0), stop=(dj == DJ - 1),
                        skip_group_check=True)
                if dj1 == DJ:
                    nc.vector.tensor_scalar(
                        out=qT_sb[:, m, qb * 512:(qb + 1) * 512], in0=st["ps"][:],
                        scalar1=bq_sb[:, m:m + 1], scalar2=None, op0=ALU.add)

            def push_qproj_fillers(m, qb):
                st = {}
                for q in range(4):
                    fillers.append((f"qp{m}{qb}", (lambda m=m, qb=qb, q=q, st=st:
                                                   qproj_mms(m, qb, 2 * q, 2 * q + 2, st))))

            # ---- out-projection (as fillers), per (q-chunk, n-half) -------
            op_stage = {}

            def outproj_piece(qc, n2):
                if qc not in op_stage:
                    op_stage[qc] = op.tile([P, D], BF16, tag="o", name=f"og{qc}")
                stage = op_stage[qc]
                ps = psC.tile([P, 512], F32, tag="pj", name=uname("ops"))
                for m in range(MC):
                    nc.tensor.matmul(
                        ps[:],
                        lhsT=ctxN[:, m, qc * P:(qc + 1) * P],
                        rhs=wo_sb[:, m, n2 * 512:(n2 + 1) * 512],
                        start=(m == 0), stop=(m == MC - 1),
                        skip_group_check=True)
                if qc >= (NQB - 1) * 4 and n2 == 1:
                    nc.scalar.copy(out=stage[:, n2 * 512:(n2 + 1) * 512], in_=ps[:])
                else:
                    nc.vector.tensor_copy(out=stage[:, n2 * 512:(n2 + 1) * 512], in_=ps[:])
                if n2 == 1:
                    nc.sync.dma_start(out=out.ap()[qc * P:(qc + 1) * P, :], in_=stage[:])
                    del op_stage[qc]

            def push_outproj_fillers(qb):
                for qc in range(qb * 4, qb * 4 + 4):
                    for n2 in range(2):
                        fillers.append(("op", lambda qc=qc, n2=n2: outproj_piece(qc, n2)))

            # ---- attention ------------------------------------------------
            # Pending AV is global so the one-group software skew spans pair
            # boundaries with no ACT gap at the seams.
            pend = {"av": None}

            def emit_av(p, qb, j, ex, ctx_ps, last):
                drain_groups({f"vp{j}"})  # vaug[j] producers must precede
                for hh in range(2):
                    nc.tensor.matmul(
                        ctx_ps[0:DV + 1, hh * 512:hh * 512 + 512],
                        lhsT=vaug[:, j, 2 * p + hh, :],
                        rhs=ex[:, hh * 512:(hh + 1) * 512],
                        start=(j == 0), stop=(j == NJ - 1),
                        skip_group_check=True)
                if last:
                    finish_pair(p, qb, ctx_ps)

            def flush_av():
                if pend["av"] is not None:
                    fn = pend["av"]
                    pend["av"] = None
                    fn()

            def emit_attention(p, qb):
                # safety: inputs of this block must already be emitted
                drain_groups({f"kp{p}", f"qp{p}{qb}"})
                q0 = qb * QB
                ctx_ps = psB.tile([P, 1024], F32, tag="ctx", name=f"ctx{p}{qb}")
                for j in range(NJ):
                    st = psA.tile([P, 1024], F32, tag="st", name=f"st{p}{qb}{j}")
                    for hh in range(2):
                        po = hh * 64
                        nc.tensor.matmul(
                            st[:, hh * 512:(hh + 1) * 512],
                            lhsT=kT_sb[po:po + 64, p, j * P:(j + 1) * P],
                            rhs=qT_sb[po:po + 64, p, q0:q0 + 512],
                            start=True, stop=True)
                    ex = ep.tile([P, 1024], BF16, tag="e", name=f"ex{p}{qb}{j}")
                    nc.scalar.activation(out=ex[:], in_=st[:], func=AF.Exp)
                    k = 2 if len(fillers) > 10 else 1
                    if pend["av"] is not None and getattr(pend["av"], "last", False):
                        # release the ctx PSUM buffer ASAP at pair seams
                        flush_av()
                        drain(k)
                    else:
                        drain(k)
                        flush_av()
                    fn = (lambda p=p, qb=qb, j=j, ex=ex, ctx_ps=ctx_ps,
                          last=(j == NJ - 1): emit_av(p, qb, j, ex, ctx_ps, last))
                    fn.last = (j == NJ - 1)
                    pend["av"] = fn

            # ---- finish a (pair, qb): evacuate ctx, normalize -------------
            def finish_pair(p, qb, ctx_ps):
                q0 = qb * QB
                ctxU = cu.tile([P, 1024], F32, tag="cu", name=f"cu{p}{qb}")
                nc.vector.tensor_copy(out=ctxU[0:DV + 1, :], in_=ctx_ps[0:DV + 1, :])
                # reciprocal of the 1024 denominators via a [128, 8] reshape
                # (single-partition DVE reciprocal is ~13us); DRAM bounces
                # do the reshape; all hops on the gpsimd DMA queue.
                rb = dscr.tile([1, 1024], F32, tag="rb")
                nc.gpsimd.dma_start(out=rb[:], in_=ctxU[DV:DV + 1, :])
                rsq = smalls.tile([P, 8], F32, tag="rsq")
                nc.gpsimd.dma_start(out=rsq[:], in_=rb.rearrange("o (p a) -> (o p) a", p=P))
                rcq = smalls.tile([P, 8], F32, tag="rcq")
                nc.vector.reciprocal(out=rcq[:], in_=rsq[:])
                rb2 = dscr.tile([1, 1024], F32, tag="rb2")
                nc.gpsimd.dma_start(out=rb2.rearrange("o (p a) -> (o p) a", p=P), in_=rcq[:])
                recb = scr.tile([P, 1024], F32, tag="s", name=f"rc{p}{qb}")
                nc.gpsimd.dma_start(out=recb[0:64, :],
                                    in_=rb2[0][None, :].partition_broadcast(64))
                # head 2p (even -> partitions 0-63) straight into ctxN
                nc.vector.tensor_tensor(
                    out=ctxN[0:64, p, q0:q0 + QB],
                    in0=ctxU[0:64, 0:512], in1=recb[0:64, 0:512], op=ALU.mult)
                # head 2p+1 (odd -> partitions 64-127) via SB->SB DMA shift
                tmp = scr.tile([P, 1024], BF16, tag="s", name=f"tm{p}{qb}")
                nc.vector.tensor_tensor(
                    out=tmp[0:64, 0:512],
                    in0=ctxU[0:64, 512:1024], in1=recb[0:64, 512:1024], op=ALU.mult)
                nc.sync.dma_start(out=ctxN[64:128, p, q0:q0 + QB], in_=tmp[0:64, 0:512])
                if p == 1:
                    push_outproj_fillers(qb)

            # ---- schedule -------------------------------------------------
            kproj_mms(0, 0, 0, DJ, {})
            for j in range(min(5, NJ)):
                vproj_mms(j, 0, DJ, {})
            qproj_mms(0, 0, 0, DJ, {})

            for kb in range(1, NKB):
                st0 = {}
                for q in range(4):
                    fillers.append(("kp0", (lambda kb=kb, q=q, st0=st0:
                                            kproj_mms(0, kb, 2 * q, 2 * q + 2, st0))))
            for j in range(5, NJ):
                push_vproj_fillers(j)
            push_kproj_fillers(1)
            for qb in range(NQB):
                for m in range(MC):
                    if (m, qb) != (0, 0):
                        push_qproj_fillers(m, qb)

            for qb in range(NQB):
                emit_attention(0, qb)
                emit_attention(1, qb)
            flush_av()          # last AV + finish_pair(1, NQB-1)
            drain_all()         # out-projection of the last q-block

    nc.compile()
    return nc


def _ensure_axon_hooks():
    """bass_utils imports antenv.axon_hooks when tracing; this image's antenv
    lacks it. Provide it, backed by the ctypes NTFF hook when available."""
    import sys
    import types
    try:
        import antenv.axon_hooks  # noqa: F401
        return
    except ImportError:
        pass
    hook = None
    try:
        from trn_agent_boot.trn_boot import _ntff_profile_via_ctypes
        hook = _ntff_profile_via_ctypes("/opt/axon/libaxon_pjrt.so")
    except Exception:
        hook = None
    mod = types.ModuleType("antenv.axon_hooks")
    mod._hook = hook
    mod.get_axon_ntff_profile_hook = lambda: mod._hook
    mod.set_axon_ntff_profile_hook = lambda h: setattr(mod, "_hook", h)
    sys.modules["antenv.axon_hooks"] = mod


def kernel(Q, K, V, atte_mask_out, Wq, bq, Wk, bk, Wv, bv, Wo, bo):
    import jax  # noqa: F401  (must be imported first so the axon backend registers)
    from concourse.bass_utils import run_bass_kernel_spmd
    global LAST_RESULTS
    _ensure_axon_hooks()

    Q = np.asarray(Q); K = np.asarray(K); V = np.asarray(V)
    mask = np.asarray(atte_mask_out).reshape(B, S)
    Wq = np.asarray(Wq); Wk = np.asarray(Wk); Wv = np.asarray(Wv); Wo = np.asarray(Wo)
    bq = np.asarray(bq); bk = np.asarray(bk); bv = np.asarray(bv); bo = np.asarray(bo)

    keep = [np.flatnonzero(~mask[b]) for b in range(B)]
    n_kp = max(P, max(((len(ix) + P - 1) // P) * P for ix in keep))

    # per-batch packed bf16 tensors
    xqT, xkT, xvT, validv = [], [], [], []
    for b in range(B):
        ix = keep[b]
        xqT.append(np.ascontiguousarray(_bf16(Q[b].T)))
        kk = np.zeros((D, n_kp), np.float32)
        vv = np.zeros((D, n_kp), np.float32)
        kk[:, :len(ix)] = K[b][ix].T
        vv[:, :len(ix)] = V[b][ix].T
        xkT.append(_bf16(kk))
        xvT.append(_bf16(vv))
        va = np.zeros(n_kp, np.float32)
        va[:len(ix)] = 1.0
        validv.append(va)

    in_maps = []
    for c in range(NCORES):
        b, g = c // GROUPS, c % GROUPS
        sl = slice(g * CH, (g + 1) * CH)
        in_maps.append({
            "xq0": np.ascontiguousarray(xqT[b][:, 0:512]),
            "xqR": np.ascontiguousarray(xqT[b][:, 512:]),
            "xkT": xkT[b], "xvT": xvT[b],
            "wqP": _wpack(_bf16(Wq[sl].T / SCALE), CH),
            "wkP": _wpack(_bf16(Wk[sl].T), CH),
            "wvP": _wpack(_bf16(Wv[sl].T), CH),
            "woP": _wpack(_bf16(Wo[:, sl].T), D),
            "bq": np.ascontiguousarray(bq[sl] / SCALE, np.float32),
            "bk": np.ascontiguousarray(bk[sl], np.float32),
            "bv": np.ascontiguousarray(bv[sl], np.float32),
            "valid": validv[b],
        })

    if n_kp not in _BUILD_CACHE:
        _BUILD_CACHE[n_kp] = _build(n_kp)
    nc = _BUILD_CACHE[n_kp]

    res = run_bass_kernel_spmd(nc, in_maps, core_ids=list(range(NCORES)))
    LAST_RESULTS = res

    full = np.zeros((B, S, D), np.float32)
    full += bo.astype(np.float32)
    for c in range(NCORES):
        full[c // GROUPS] += np.asarray(res.results[c]["out"], np.float32)
    return full


# revision 13
# speedup vs baseline: 1.5778x; 1.0362x over previous
"""Multi-head attention (B=2, S=2048, D=1024, H=16, dk=dv=64) on 8 TRN2 NeuronCores.

Sharding: core c -> (batch b = c//4, head-group g = c%4, 4 heads each).
Each core computes q/k/v projections for its 4 heads (weight-column shard),
attention over its batch, and a partial output projection over its 256
channels (weight-row shard of Wo).  The host sums the 4 partial outputs per
batch at unshard time (the "all-reduce after the output projection").

v3 design: the ACT engine's exp stream is the hard lower bound
(64 x (1024+352)/1.2 ns ~= 73us, dtype-independent), so the whole kernel is
scheduled around keeping ACT saturated from ~20us on:

  * All matmul operands are bf16 (halves DMA; PE rate = fp32r at 512-wide).
  * Scores for the two heads of an m-chunk (K = dk = 64) are issued
    back-to-back as PE row-tiled matmuls (rows 0-63 / 64-127) -> they
    stream concurrently; one exp instr covers both heads [128, 1024].
  * Global software pipeline: for each score group g = (pair, qb, j):
    emit ST(g); exp(g); then <=2 "filler" PE pieces (deferred qproj/kproj/
    vproj/out-proj matmuls, 512-row granularity) from a queue; then AV(g-1).
    The PE never runs a multi-us block that would starve ACT, and never
    idles >3us (which would HAM-throttle it to 1.2 GHz).
  * Attention context is evacuated from PSUM to SBUF immediately after the
    last AV of a (pair, qb) so the single ctx PSUM buffer recycles fast;
    the softmax normalization (1/denominator from the 65th "ones" column
    of V_aug) happens from SBUF off the critical path.
  * Key-padding mask applied by host-side COMPACTION of K/V; `valid`
    zeroes padded tail rows of V_aug (their exp(0)=1 x 0 adds nothing).
"""
import numpy as np

B, S, D = 2, 2048, 1024
H, DK, DV = 16, 64, 64
SCALE = float(np.sqrt(DK))
NCORES = 8
GROUPS = 4           # head-groups (cores per batch)
HPG = H // GROUPS    # heads per core = 4
CH = HPG * DK        # channels per core = 256
MC = CH // 128       # m-chunks = head-pairs = 2
DJ = D // 128        # contraction chunks = 8
P = 128
QB = 512             # q-block width
NQB = S // QB        # 4

_BUILD_CACHE = {}
LAST_RESULTS = None  # test harness can read exec_time_ns etc. from here


def _bf16(a: np.ndarray):
    import ml_dtypes
    return np.ascontiguousarray(np.asarray(a, np.float32)).astype(ml_dtypes.bfloat16)


def _wpack(wT: np.ndarray, cols: int) -> np.ndarray:
    """[J*128, cols] -> [128, J*cols]: row j*128+p lands at [p, j, :]."""
    J = wT.shape[0] // 128
    return np.ascontiguousarray(
        wT.reshape(J, 128, cols).transpose(1, 0, 2).reshape(128, J * cols))


def _build(n_kp: int):
    """Build + schedule the per-core Bass program for a padded key count."""
    import concourse.bass as bass  # noqa: F401
    from concourse import bacc, tile, mybir
    from collections import deque

    DT = mybir.dt
    F32, BF16 = DT.float32, DT.bfloat16
    AF = mybir.ActivationFunctionType
    ALU = mybir.AluOpType

    NJ = n_kp // P                      # k-chunks
    NKB = (n_kp + 511) // 512           # 512-wide k blocks for the k projection

    nc = bacc.Bacc("TRN2", target_bir_lowering=False, debug=False,
                   num_devices=NCORES)

    xkT = nc.dram_tensor("xkT", [D, n_kp], BF16, kind="ExternalInput")
    xvT = nc.dram_tensor("xvT", [D, n_kp], BF16, kind="ExternalInput")
    # weights arrive host-pre-shuffled so each is ONE contiguous DMA:
    # wxP[p, dj, c] = W.T[dj*128+p, c]; woP[p, m, d] = Wo.T[m*128+p, d]
    wqP = nc.dram_tensor("wqP", [P, DJ * CH], BF16, kind="ExternalInput")
    wkP = nc.dram_tensor("wkP", [P, DJ * CH], BF16, kind="ExternalInput")
    wvP = nc.dram_tensor("wvP", [P, DJ * CH], BF16, kind="ExternalInput")
    woP = nc.dram_tensor("woP", [P, MC * D], BF16, kind="ExternalInput")
    xq0 = nc.dram_tensor("xq0", [D, 512], BF16, kind="ExternalInput")
    xqR = nc.dram_tensor("xqR", [D, S - 512], BF16, kind="ExternalInput")
    bq = nc.dram_tensor("bq", [CH], F32, kind="ExternalInput")
    bk = nc.dram_tensor("bk", [CH], F32, kind="ExternalInput")
    bv = nc.dram_tensor("bv", [CH], F32, kind="ExternalInput")
    valid = nc.dram_tensor("valid", [n_kp], F32, kind="ExternalInput")
    out = nc.dram_tensor("out", [S, D], BF16, kind="ExternalOutput")

    with tile.TileContext(nc) as tc:
        with (
            tc.tile_pool(name="xs", bufs=3 * DJ) as xs,
            tc.tile_pool(name="persist", bufs=1) as pp,
            tc.tile_pool(name="exps", bufs=4) as ep,
            tc.tile_pool(name="scratch", bufs=4) as scr,
            tc.tile_pool(name="cu", bufs=2) as cu,
            tc.tile_pool(name="outs", bufs=3) as op,
            tc.tile_pool(name="smalls", bufs=4) as smalls,
            tc.tile_pool(name="psA", bufs=2, space="PSUM") as psA,
            tc.tile_pool(name="psB", bufs=1, space="PSUM") as psB,
            tc.tile_pool(name="psC", bufs=2, space="PSUM") as psC,
            tc.tile_pool(name="dscr", bufs=3, space="DRAM") as dscr,
        ):
            # ---- persistent SBUF ------------------------------------------
            wq_sb = pp.tile([P, DJ, CH], BF16, name="wq_sb")
            wk_sb = pp.tile([P, DJ, CH], BF16, name="wk_sb")
            wv_sb = pp.tile([P, DJ, CH], BF16, name="wv_sb")
            wo_sb = pp.tile([P, MC, D], BF16, name="wo_sb")
            bq_sb = pp.tile([P, MC], F32, name="bq_sb")
            bk_sb = pp.tile([P, MC], F32, name="bk_sb")
            qT_sb = pp.tile([P, MC, S], BF16, name="qT_sb")
            kT_sb = pp.tile([P, MC, n_kp], BF16, name="kT_sb")
            vaug = pp.tile([P, NJ, HPG, DV + 1], BF16, name="vaug")
            ctxN = pp.tile([P, MC, S], BF16, name="ctxN")
            bv_rep = pp.tile([P, CH], F32, name="bv_rep")
            valid_sb = pp.tile([P, NJ], F32, name="valid_sb")
            valid_bf = pp.tile([P, NJ], BF16, name="valid_bf")

            # ---- DMA preamble, split across engine queues -----------------
            # sync queue: k inputs (first PE work), then q inputs (qb0
            # columns first so attention can start), then the q remainder.
            xk_t = [xs.tile([P, S], BF16, tag="x", name=f"xk{dj}") for dj in range(DJ)]
            xv_t = [xs.tile([P, S], BF16, tag="x", name=f"xv{dj}") for dj in range(DJ)]
            xq_t = [xs.tile([P, S], BF16, tag="x", name=f"xq{dj}") for dj in range(DJ)]
            # Per-queue DMA transfers serialize (~110 GB/s each), so the
            # 6.5MB needed before the first exp is spread over 4 queues,
            # ordered by when the PE consumes it:
            #   sync:   xk[0:4], xv[4:8], xqR[0:4], (later) shifts + out
            #   scalar: wk, xk[4:8], wv, wq, xqR[4:8], wo
            #   gpsimd: consts, xv[0:4], (later) normalize hops
            #   gpsimd (cont.): xq0
            for dj in range(4):
                nc.sync.dma_start(out=xk_t[dj][:, :n_kp], in_=xkT.ap()[dj * P:(dj + 1) * P, :])
            nc.scalar.dma_start(out=wk_sb.rearrange("p j c -> p (j c)"), in_=wkP.ap())
            for dj in range(4, DJ):
                nc.scalar.dma_start(out=xk_t[dj][:, :n_kp], in_=xkT.ap()[dj * P:(dj + 1) * P, :])
            nc.gpsimd.dma_start(out=bk_sb[:], in_=bk.ap().rearrange("(m p) -> p m", p=P))
            nc.gpsimd.dma_start(out=bq_sb[:], in_=bq.ap().rearrange("(m p) -> p m", p=P))
            nc.gpsimd.dma_start(out=bv_rep[:], in_=bv.ap()[None, :].partition_broadcast(P))
            nc.gpsimd.dma_start(out=valid_sb[:], in_=valid.ap().rearrange("(j p) -> p j", p=P))
            for dj in range(4):
                nc.gpsimd.dma_start(out=xv_t[dj][:, :n_kp], in_=xvT.ap()[dj * P:(dj + 1) * P, :])
            for dj in range(4, DJ):
                nc.sync.dma_start(out=xv_t[dj][:, :n_kp], in_=xvT.ap()[dj * P:(dj + 1) * P, :])
            for dj in range(DJ):
                nc.gpsimd.dma_start(out=xq_t[dj][:, 0:QB], in_=xq0.ap()[dj * P:(dj + 1) * P, :])
            nc.scalar.dma_start(out=wv_sb.rearrange("p j c -> p (j c)"), in_=wvP.ap())
            nc.scalar.dma_start(out=wq_sb.rearrange("p j c -> p (j c)"), in_=wqP.ap())
            for dj in range(4):
                nc.sync.dma_start(out=xq_t[dj][:, QB:], in_=xqR.ap()[dj * P:(dj + 1) * P, :])
            for dj in range(4, DJ):
                nc.scalar.dma_start(out=xq_t[dj][:, QB:], in_=xqR.ap()[dj * P:(dj + 1) * P, :])
            nc.scalar.dma_start(out=wo_sb.rearrange("p m d -> p (m d)"), in_=woP.ap())
            nc.vector.tensor_copy(out=valid_bf[:], in_=valid_sb[:])

            # ---- filler queue machinery -----------------------------------
            fillers = deque()   # (group, closure) - ~0.5us of PE work each
            _uid = [0]

            def uname(pfx):
                _uid[0] += 1
                return f"{pfx}{_uid[0]}"

            def drain(n):
                for _ in range(min(n, len(fillers))):
                    g, fn = fillers.popleft()
                    fn()

            def drain_groups(groups):
                """Emit every queued filler belonging to `groups` (and
                anything queued ahead of them - FIFO order preserved)."""
                while any(g in groups for g, _ in fillers):
                    g, fn = fillers.popleft()
                    fn()

            def drain_all():
                while fillers:
                    fillers.popleft()[1]()

            # ---- k projection ---------------------------------------------
            def kproj_mms(m, kb, dj0, dj1, st):
                if "ps" not in st:
                    st["ps"] = psC.tile([P, 512], F32, tag="pj", name=uname("kps"))
                w = min(512, n_kp - kb * 512)
                for dj in range(dj0, dj1):
                    nc.tensor.matmul(
                        st["ps"][:, :w],
                        lhsT=wk_sb[:, dj, m * P:(m + 1) * P],
                        rhs=xk_t[dj][:, kb * 512:kb * 512 + w],
                        start=(dj == 0), stop=(dj == DJ - 1),
                        skip_group_check=True)
                if dj1 == DJ:
                    nc.vector.tensor_scalar(
                        out=kT_sb[:, m, kb * 512:kb * 512 + w], in0=st["ps"][:, :w],
                        scalar1=bk_sb[:, m:m + 1], scalar2=None, op0=ALU.add)

            def emit_kproj(m):
                for kb in range(NKB):
                    kproj_mms(m, kb, 0, DJ, {})

            def push_kproj_fillers(m):
                for kb in range(NKB):
                    st = {}
                    for q in range(4):
                        fillers.append((f"kp{m}", (lambda kb=kb, q=q, st=st:
                                                   kproj_mms(m, kb, 2 * q, 2 * q + 2, st))))

            # ---- v projection ---------------------------------------------
            def vproj_mms(j, dj0, dj1, st):
                if "ps" not in st:
                    st["ps"] = psC.tile([P, 512], F32, tag="pj", name=uname("vps"))
                ps = st["ps"]
                for dj in range(dj0, dj1):
                    nc.tensor.matmul(
                        ps[:, :CH],
                        lhsT=xv_t[dj][:, j * P:(j + 1) * P],
                        rhs=wv_sb[:, dj, :],
                        start=(dj == 0), stop=(dj == DJ - 1),
                        skip_group_check=True)
                if dj1 == DJ:
                    vst = scr.tile([P, 1024], F32, tag="s", name=uname("vst"))
                    nc.vector.tensor_tensor(out=vst[:, :CH], in0=ps[:, :CH],
                                            in1=bv_rep[:], op=ALU.add)
                    nc.vector.tensor_scalar(
                        out=vaug[:, j, :, 0:DV],
                        in0=vst[:, :CH].rearrange("p (h d) -> p h d", h=HPG),
                        scalar1=valid_sb[:, j:j + 1], scalar2=None, op0=ALU.mult)
                    for h in range(HPG):
                        nc.gpsimd.tensor_copy(out=vaug[:, j, h, DV:DV + 1],
                                              in_=valid_bf[:, j:j + 1])

            def push_vproj_fillers(j):
                st = {}
                fillers.append((f"vp{j}", lambda j=j, st=st: vproj_mms(j, 0, 4, st)))
                fillers.append((f"vp{j}", lambda j=j, st=st: vproj_mms(j, 4, DJ, st)))

            # ---- q projection ---------------------------------------------
            def qproj_mms(m, qb, dj0, dj1, st):
                if "ps" not in st:
                    st["ps"] = psC.tile([P, 512], F32, tag="pj", name=uname("qps"))
                for dj in range(dj0, dj1):
                    nc.tensor.matmul(
                        st["ps"][:],
                        lhsT=wq_sb[:, dj, m * P:(m + 1) * P],
                        rhs=xq_t[dj][:, qb * 512:(qb + 1) * 512],
                        start=(dj == 0), stop=(dj == DJ - 1),
                        skip_group_check=True)
                if dj1 == DJ:
                    nc.vector.tensor_scalar(
                        out=qT_sb[:, m, qb * 512:(qb + 1) * 512], in0=st["ps"][:],
                        scalar1=bq_sb[:, m:m + 1], scalar2=None, op0=ALU.add)

            def push_qproj_fillers(m, qb):
                st = {}
                for q in range(4):
                    fillers.append((f"qp{m}{qb}", (lambda m=m, qb=qb, q=q, st=st:
                                                   qproj_mms(m, qb, 2 * q, 2 * q + 2, st))))

            # ---- out-projection (as fillers), per (q-chunk, n-half) -------
            op_stage = {}

            def outproj_piece(qc, n2):
                if qc not in op_stage:
                    op_stage[qc] = op.tile([P, D], BF16, tag="o", name=f"og{qc}")
                stage = op_stage[qc]
                ps = psC.tile([P, 512], F32, tag="pj", name=uname("ops"))
                for m in range(MC):
                    nc.tensor.matmul(
                        ps[:],
                        lhsT=ctxN[:, m, qc * P:(qc + 1) * P],
                        rhs=wo_sb[:, m, n2 * 512:(n2 + 1) * 512],
                        start=(m == 0), stop=(m == MC - 1),
                        skip_group_check=True)
                if qc >= (NQB - 1) * 4:
                    nc.scalar.copy(out=stage[:, n2 * 512:(n2 + 1) * 512], in_=ps[:])
                else:
                    nc.vector.tensor_copy(out=stage[:, n2 * 512:(n2 + 1) * 512], in_=ps[:])
                if n2 == 1:
                    nc.sync.dma_start(out=out.ap()[qc * P:(qc + 1) * P, :], in_=stage[:])
                    del op_stage[qc]

            def push_outproj_fillers(qb):
                for qc in range(qb * 4, qb * 4 + 4):
                    for n2 in range(2):
                        fillers.append(("op", lambda qc=qc, n2=n2: outproj_piece(qc, n2)))

            # ---- attention ------------------------------------------------
            # Pending AV is global so the one-group software skew spans pair
            # boundaries with no ACT gap at the seams.
            pend = {"av": None}

            def emit_av(p, qb, j, ex, ctx_ps, last):
                drain_groups({f"vp{j}"})  # vaug[j] producers must precede
                for hh in range(2):
                    nc.tensor.matmul(
                        ctx_ps[0:DV + 1, hh * 512:hh * 512 + 512],
                        lhsT=vaug[:, j, 2 * p + hh, :],
                        rhs=ex[:, hh * 512:(hh + 1) * 512],
                        start=(j == 0), stop=(j == NJ - 1),
                        skip_group_check=True)
                if last:
                    finish_pair(p, qb, ctx_ps)

            def flush_av():
                if pend["av"] is not None:
                    fn = pend["av"]
                    pend["av"] = None
                    fn()

            def emit_attention(p, qb):
                # safety: inputs of this block must already be emitted
                drain_groups({f"kp{p}", f"qp{p}{qb}"})
                q0 = qb * QB
                ctx_ps = psB.tile([P, 1024], F32, tag="ctx", name=f"ctx{p}{qb}")
                for j in range(NJ):
                    st = psA.tile([P, 1024], F32, tag="st", name=f"st{p}{qb}{j}")
                    for hh in range(2):
                        po = hh * 64
                        nc.tensor.matmul(
                            st[:, hh * 512:(hh + 1) * 512],
                            lhsT=kT_sb[po:po + 64, p, j * P:(j + 1) * P],
                            rhs=qT_sb[po:po + 64, p, q0:q0 + 512],
                            start=True, stop=True)
                    ex = ep.tile([P, 1024], BF16, tag="e", name=f"ex{p}{qb}{j}")
                    nc.scalar.activation(out=ex[:], in_=st[:], func=AF.Exp)
                    k = 2 if len(fillers) > 10 else 1
                    if pend["av"] is not None and getattr(pend["av"], "last", False):
                        # release the ctx PSUM buffer ASAP at pair seams
                        flush_av()
                        drain(k)
                    else:
                        drain(k)
                        flush_av()
                    fn = (lambda p=p, qb=qb, j=j, ex=ex, ctx_ps=ctx_ps,
                          last=(j == NJ - 1): emit_av(p, qb, j, ex, ctx_ps, last))
                    fn.last = (j == NJ - 1)
                    pend["av"] = fn

            # ---- finish a (pair, qb): evacuate ctx, normalize -------------
            def finish_pair(p, qb, ctx_ps):
                q0 = qb * QB
                ctxU = cu.tile([P, 1024], F32, tag="cu", name=f"cu{p}{qb}")
                nc.vector.tensor_copy(out=ctxU[0:DV + 1, :], in_=ctx_ps[0:DV + 1, :])
                # reciprocal of the 1024 denominators via a [128, 8] reshape
                # (single-partition DVE reciprocal is ~13us); DRAM bounces
                # do the reshape; all hops on the gpsimd DMA queue.
                rb = dscr.tile([1, 1024], F32, tag="rb")
                nc.gpsimd.dma_start(out=rb[:], in_=ctxU[DV:DV + 1, :])
                rsq = smalls.tile([P, 8], F32, tag="rsq")
                nc.gpsimd.dma_start(out=rsq[:], in_=rb.rearrange("o (p a) -> (o p) a", p=P))
                rcq = smalls.tile([P, 8], F32, tag="rcq")
                nc.vector.reciprocal(out=rcq[:], in_=rsq[:])
                rb2 = dscr.tile([1, 1024], F32, tag="rb2")
                nc.gpsimd.dma_start(out=rb2.rearrange("o (p a) -> (o p) a", p=P), in_=rcq[:])
                recb = scr.tile([P, 1024], F32, tag="s", name=f"rc{p}{qb}")
                nc.gpsimd.dma_start(out=recb[0:64, :],
                                    in_=rb2[0][None, :].partition_broadcast(64))
                # head 2p (even -> partitions 0-63) straight into ctxN
                nc.vector.tensor_tensor(
                    out=ctxN[0:64, p, q0:q0 + QB],
                    in0=ctxU[0:64, 0:512], in1=recb[0:64, 0:512], op=ALU.mult)
                # head 2p+1 (odd -> partitions 64-127) via SB->SB DMA shift
                tmp = scr.tile([P, 1024], BF16, tag="s", name=f"tm{p}{qb}")
                nc.vector.tensor_tensor(
                    out=tmp[0:64, 0:512],
                    in0=ctxU[0:64, 512:1024], in1=recb[0:64, 512:1024], op=ALU.mult)
                nc.sync.dma_start(out=ctxN[64:128, p, q0:q0 + QB], in_=tmp[0:64, 0:512])
                if p == 1:
                    push_outproj_fillers(qb)

            # ---- schedule -------------------------------------------------
            kproj_mms(0, 0, 0, DJ, {})
            for j in range(min(5, NJ)):
                vproj_mms(j, 0, DJ, {})
            qproj_mms(0, 0, 0, DJ, {})

            for kb in range(1, NKB):
                st0 = {}
                for q in range(4):
                    fillers.append(("kp0", (lambda kb=kb, q=q, st0=st0:
                                            kproj_mms(0, kb, 2 * q, 2 * q + 2, st0))))
            for j in range(5, NJ):
                push_vproj_fillers(j)
            push_kproj_fillers(1)
            for qb in range(NQB):
                for m in range(MC):
                    if (m, qb) != (0, 0):
                        push_qproj_fillers(m, qb)

            for qb in range(NQB):
                emit_attention(0, qb)
                emit_attention(1, qb)
            flush_av()          # last AV + finish_pair(1, NQB-1)
            drain_all()         # out-projection of the last q-block

    nc.compile()
    return nc


def _ensure_axon_hooks():
    """bass_utils imports antenv.axon_hooks when tracing; this image's antenv
    lacks it. Provide it, backed by the ctypes NTFF hook when available."""
    import sys
    import types
    try:
        import antenv.axon_hooks  # noqa: F401
        return
    except ImportError:
        pass
    hook = None
    try:
        from trn_agent_boot.trn_boot import _ntff_profile_via_ctypes
        hook = _ntff_profile_via_ctypes("/opt/axon/libaxon_pjrt.so")
    except Exception:
        hook = None
    mod = types.ModuleType("antenv.axon_hooks")
    mod._hook = hook
    mod.get_axon_ntff_profile_hook = lambda: mod._hook
    mod.set_axon_ntff_profile_hook = lambda h: setattr(mod, "_hook", h)
    sys.modules["antenv.axon_hooks"] = mod


def kernel(Q, K, V, atte_mask_out, Wq, bq, Wk, bk, Wv, bv, Wo, bo):
    import jax  # noqa: F401  (must be imported first so the axon backend registers)
    from concourse.bass_utils import run_bass_kernel_spmd
    global LAST_RESULTS
    _ensure_axon_hooks()

    Q = np.asarray(Q); K = np.asarray(K); V = np.asarray(V)
    mask = np.asarray(atte_mask_out).reshape(B, S)
    Wq = np.asarray(Wq); Wk = np.asarray(Wk); Wv = np.asarray(Wv); Wo = np.asarray(Wo)
    bq = np.asarray(bq); bk = np.asarray(bk); bv = np.asarray(bv); bo = np.asarray(bo)

    keep = [np.flatnonzero(~mask[b]) for b in range(B)]
    n_kp = max(P, max(((len(ix) + P - 1) // P) * P for ix in keep))

    # per-batch packed bf16 tensors
    xqT, xkT, xvT, validv = [], [], [], []
    for b in range(B):
        ix = keep[b]
        xqT.append(np.ascontiguousarray(_bf16(Q[b].T)))
        kk = np.zeros((D, n_kp), np.float32)
        vv = np.zeros((D, n_kp), np.float32)
        kk[:, :len(ix)] = K[b][ix].T
        vv[:, :len(ix)] = V[b][ix].T
        xkT.append(_bf16(kk))
        xvT.append(_bf16(vv))
        va = np.zeros(n_kp, np.float32)
        va[:len(ix)] = 1.0
        validv.append(va)

    in_maps = []
    for c in range(NCORES):
        b, g = c // GROUPS, c % GROUPS
        sl = slice(g * CH, (g + 1) * CH)
        in_maps.append({
            "xq0": np.ascontiguousarray(xqT[b][:, 0:512]),
            "xqR": np.ascontiguousarray(xqT[b][:, 512:]),
            "xkT": xkT[b], "xvT": xvT[b],
            "wqP": _wpack(_bf16(Wq[sl].T / SCALE), CH),
            "wkP": _wpack(_bf16(Wk[sl].T), CH),
            "wvP": _wpack(_bf16(Wv[sl].T), CH),
            "woP": _wpack(_bf16(Wo[:, sl].T), D),
            "bq": np.ascontiguousarray(bq[sl] / SCALE, np.float32),
            "bk": np.ascontiguousarray(bk[sl], np.float32),
            "bv": np.ascontiguousarray(bv[sl], np.float32),
            "valid": validv[b],
        })

    if n_kp not in _BUILD_CACHE:
        _BUILD_CACHE[n_kp] = _build(n_kp)
    nc = _BUILD_CACHE[n_kp]

    res = run_bass_kernel_spmd(nc, in_maps, core_ids=list(range(NCORES)))
    LAST_RESULTS = res

    full = np.zeros((B, S, D), np.float32)
    full += bo.astype(np.float32)
    for c in range(NCORES):
        full[c // GROUPS] += np.asarray(res.results[c]["out"], np.float32)
    return full


# revision 15
# speedup vs baseline: 1.5994x; 1.0137x over previous
"""Multi-head attention (B=2, S=2048, D=1024, H=16, dk=dv=64) on 8 TRN2 NeuronCores.

Sharding: core c -> (batch b = c//4, head-group g = c%4, 4 heads each).
Each core computes q/k/v projections for its 4 heads (weight-column shard),
attention over its batch, and a partial output projection over its 256
channels (weight-row shard of Wo).  The host sums the 4 partial outputs per
batch at unshard time (the "all-reduce after the output projection").

v3 design: the ACT engine's exp stream is the hard lower bound
(64 x (1024+352)/1.2 ns ~= 73us, dtype-independent), so the whole kernel is
scheduled around keeping ACT saturated from ~20us on:

  * All matmul operands are bf16 (halves DMA; PE rate = fp32r at 512-wide).
  * Scores for the two heads of an m-chunk (K = dk = 64) are issued
    back-to-back as PE row-tiled matmuls (rows 0-63 / 64-127) -> they
    stream concurrently; one exp instr covers both heads [128, 1024].
  * Global software pipeline: for each score group g = (pair, qb, j):
    emit ST(g); exp(g); then <=2 "filler" PE pieces (deferred qproj/kproj/
    vproj/out-proj matmuls, 512-row granularity) from a queue; then AV(g-1).
    The PE never runs a multi-us block that would starve ACT, and never
    idles >3us (which would HAM-throttle it to 1.2 GHz).
  * Attention context is evacuated from PSUM to SBUF immediately after the
    last AV of a (pair, qb) so the single ctx PSUM buffer recycles fast;
    the softmax normalization (1/denominator from the 65th "ones" column
    of V_aug) happens from SBUF off the critical path.
  * Key-padding mask applied by host-side COMPACTION of K/V; `valid`
    zeroes padded tail rows of V_aug (their exp(0)=1 x 0 adds nothing).
"""
import numpy as np

B, S, D = 2, 2048, 1024
H, DK, DV = 16, 64, 64
SCALE = float(np.sqrt(DK))
NCORES = 8
GROUPS = 4           # head-groups (cores per batch)
HPG = H // GROUPS    # heads per core = 4
CH = HPG * DK        # channels per core = 256
MC = CH // 128       # m-chunks = head-pairs = 2
DJ = D // 128        # contraction chunks = 8
P = 128
QB = 512             # q-block width
NQB = S // QB        # 4

_BUILD_CACHE = {}
LAST_RESULTS = None  # test harness can read exec_time_ns etc. from here


def _bf16(a: np.ndarray):
    import ml_dtypes
    return np.ascontiguousarray(np.asarray(a, np.float32)).astype(ml_dtypes.bfloat16)


def _wpack(wT: np.ndarray, cols: int) -> np.ndarray:
    """[J*128, cols] -> [128, J*cols]: row j*128+p lands at [p, j, :]."""
    J = wT.shape[0] // 128
    return np.ascontiguousarray(
        wT.reshape(J, 128, cols).transpose(1, 0, 2).reshape(128, J * cols))


def _build(n_kp: int):
    """Build + schedule the per-core Bass program for a padded key count."""
    import concourse.bass as bass  # noqa: F401
    from concourse import bacc, tile, mybir
    from collections import deque

    DT = mybir.dt
    F32, BF16 = DT.float32, DT.bfloat16
    AF = mybir.ActivationFunctionType
    ALU = mybir.AluOpType

    NJ = n_kp // P                      # k-chunks
    NKB = (n_kp + 511) // 512           # 512-wide k blocks for the k projection

    nc = bacc.Bacc("TRN2", target_bir_lowering=False, debug=False,
                   num_devices=NCORES)

    xkT = nc.dram_tensor("xkT", [D, n_kp], BF16, kind="ExternalInput")
    xvT = nc.dram_tensor("xvT", [D, n_kp], BF16, kind="ExternalInput")
    # weights arrive host-pre-shuffled so each is ONE contiguous DMA:
    # wxP[p, dj, c] = W.T[dj*128+p, c]; woP[p, m, d] = Wo.T[m*128+p, d]
    wqP = nc.dram_tensor("wqP", [P, DJ * CH], BF16, kind="ExternalInput")
    wkP = nc.dram_tensor("wkP", [P, DJ * CH], BF16, kind="ExternalInput")
    wvP = nc.dram_tensor("wvP", [P, DJ * CH], BF16, kind="ExternalInput")
    woP = nc.dram_tensor("woP", [P, MC * D], BF16, kind="ExternalInput")
    xq0 = nc.dram_tensor("xq0", [D, 512], BF16, kind="ExternalInput")
    xq1 = nc.dram_tensor("xq1", [D, 512], BF16, kind="ExternalInput")
    xq23 = nc.dram_tensor("xq23", [D, 1024], BF16, kind="ExternalInput")
    bq = nc.dram_tensor("bq", [CH], F32, kind="ExternalInput")
    bk = nc.dram_tensor("bk", [CH], F32, kind="ExternalInput")
    bv = nc.dram_tensor("bv", [CH], F32, kind="ExternalInput")
    valid = nc.dram_tensor("valid", [n_kp], F32, kind="ExternalInput")
    out = nc.dram_tensor("out", [S, D], BF16, kind="ExternalOutput")

    with tile.TileContext(nc) as tc:
        with (
            tc.tile_pool(name="xs", bufs=3 * DJ) as xs,
            tc.tile_pool(name="persist", bufs=1) as pp,
            tc.tile_pool(name="exps", bufs=4) as ep,
            tc.tile_pool(name="scratch", bufs=4) as scr,
            tc.tile_pool(name="cu", bufs=2) as cu,
            tc.tile_pool(name="outs", bufs=3) as op,
            tc.tile_pool(name="smalls", bufs=4) as smalls,
            tc.tile_pool(name="psA", bufs=2, space="PSUM") as psA,
            tc.tile_pool(name="psB", bufs=1, space="PSUM") as psB,
            tc.tile_pool(name="psC", bufs=2, space="PSUM") as psC,
            tc.tile_pool(name="dscr", bufs=3, space="DRAM") as dscr,
        ):
            # ---- persistent SBUF ------------------------------------------
            wq_sb = pp.tile([P, DJ, CH], BF16, name="wq_sb")
            wk_sb = pp.tile([P, DJ, CH], BF16, name="wk_sb")
            wv_sb = pp.tile([P, DJ, CH], BF16, name="wv_sb")
            wo_sb = pp.tile([P, MC, D], BF16, name="wo_sb")
            bq_sb = pp.tile([P, MC], F32, name="bq_sb")
            bk_sb = pp.tile([P, MC], F32, name="bk_sb")
            qT_sb = pp.tile([P, MC, S], BF16, name="qT_sb")
            kT_sb = pp.tile([P, MC, n_kp], BF16, name="kT_sb")
            vaug = pp.tile([P, NJ, HPG, DV + 1], BF16, name="vaug")
            ctxN = pp.tile([P, MC, S], BF16, name="ctxN")
            bv_rep = pp.tile([P, CH], F32, name="bv_rep")
            valid_sb = pp.tile([P, NJ], F32, name="valid_sb")
            valid_bf = pp.tile([P, NJ], BF16, name="valid_bf")

            # warmup operand: zeroed early on the (idle) DVE queue so the
            # first dummy matmul can issue as soon as the PE boots.
            dum = pp.tile([P, 512], BF16, name="dum")
            nc.vector.memset(dum[:], 0.0)

            # ---- DMA preamble, split across engine queues -----------------
            # sync queue: k inputs (first PE work), then q inputs (qb0
            # columns first so attention can start), then the q remainder.
            xk_t = [xs.tile([P, S], BF16, tag="x", name=f"xk{dj}") for dj in range(DJ)]
            xv_t = [xs.tile([P, S], BF16, tag="x", name=f"xv{dj}") for dj in range(DJ)]
            xq_t = [xs.tile([P, S], BF16, tag="x", name=f"xq{dj}") for dj in range(DJ)]
            # Per-queue DMA transfers serialize (~110 GB/s each), so the
            # preamble is round-robined across the 3 DMA-capable queues
            # (sync/scalar/gpsimd) in PE-consumption order.
            nc.gpsimd.dma_start(out=bk_sb[:], in_=bk.ap().rearrange("(m p) -> p m", p=P))
            nc.gpsimd.dma_start(out=bq_sb[:], in_=bq.ap().rearrange("(m p) -> p m", p=P))
            nc.gpsimd.dma_start(out=bv_rep[:], in_=bv.ap()[None, :].partition_broadcast(P))
            nc.gpsimd.dma_start(out=valid_sb[:], in_=valid.ap().rearrange("(j p) -> p j", p=P))
            _qs = [nc.sync, nc.scalar, nc.gpsimd]
            _qi = [0]

            def q_dma(out, in_):
                _qs[_qi[0] % 3].dma_start(out=out, in_=in_)
                _qi[0] += 1

            q_dma(wk_sb.rearrange("p j c -> p (j c)"), wkP.ap())
            for dj in range(DJ):
                q_dma(xk_t[dj][:, :n_kp], xkT.ap()[dj * P:(dj + 1) * P, :])
            q_dma(wv_sb.rearrange("p j c -> p (j c)"), wvP.ap())
            for dj in range(DJ):
                q_dma(xv_t[dj][:, :n_kp], xvT.ap()[dj * P:(dj + 1) * P, :])
            q_dma(wq_sb.rearrange("p j c -> p (j c)"), wqP.ap())
            for dj in range(DJ):
                q_dma(xq_t[dj][:, 0:QB], xq0.ap()[dj * P:(dj + 1) * P, :])
            for dj in range(DJ):
                q_dma(xq_t[dj][:, QB:2 * QB], xq1.ap()[dj * P:(dj + 1) * P, :])
            q_dma(wo_sb.rearrange("p m d -> p (m d)"), woP.ap())
            for dj in range(DJ):
                q_dma(xq_t[dj][:, 2 * QB:], xq23.ap()[dj * P:(dj + 1) * P, :])
            nc.vector.tensor_copy(out=valid_bf[:], in_=valid_sb[:])

            # ---- filler queue machinery -----------------------------------
            fillers = deque()   # (group, closure) - ~0.5us of PE work each
            _uid = [0]

            def uname(pfx):
                _uid[0] += 1
                return f"{pfx}{_uid[0]}"

            def drain(n):
                for _ in range(min(n, len(fillers))):
                    g, fn = fillers.popleft()
                    fn()

            def drain_groups(groups):
                """Emit every queued filler belonging to `groups` (and
                anything queued ahead of them - FIFO order preserved)."""
                while any(g in groups for g, _ in fillers):
                    g, fn = fillers.popleft()
                    fn()

            def drain_all():
                while fillers:
                    fillers.popleft()[1]()

            # ---- k projection ---------------------------------------------
            def kproj_mms(m, kb, dj0, dj1, st):
                if "ps" not in st:
                    st["ps"] = psC.tile([P, 512], F32, tag="pj", name=uname("kps"))
                w = min(512, n_kp - kb * 512)
                for dj in range(dj0, dj1):
                    nc.tensor.matmul(
                        st["ps"][:, :w],
                        lhsT=wk_sb[:, dj, m * P:(m + 1) * P],
                        rhs=xk_t[dj][:, kb * 512:kb * 512 + w],
                        start=(dj == 0), stop=(dj == DJ - 1),
                        skip_group_check=True)
                if dj1 == DJ:
                    nc.vector.tensor_scalar(
                        out=kT_sb[:, m, kb * 512:kb * 512 + w], in0=st["ps"][:, :w],
                        scalar1=bk_sb[:, m:m + 1], scalar2=None, op0=ALU.add)

            def emit_kproj(m):
                for kb in range(NKB):
                    kproj_mms(m, kb, 0, DJ, {})

            def push_kproj_fillers(m):
                for kb in range(NKB):
                    st = {}
                    for q in range(4):
                        fillers.append((f"kp{m}", (lambda kb=kb, q=q, st=st:
                                                   kproj_mms(m, kb, 2 * q, 2 * q + 2, st))))

            # ---- v projection ---------------------------------------------
            def vproj_mms(j, dj0, dj1, st):
                if "ps" not in st:
                    st["ps"] = psC.tile([P, 512], F32, tag="pj", name=uname("vps"))
                ps = st["ps"]
                for dj in range(dj0, dj1):
                    nc.tensor.matmul(
                        ps[:, :CH],
                        lhsT=xv_t[dj][:, j * P:(j + 1) * P],
                        rhs=wv_sb[:, dj, :],
                        start=(dj == 0), stop=(dj == DJ - 1),
                        skip_group_check=True)
                if dj1 == DJ:
                    vst = scr.tile([P, 1024], F32, tag="s", name=uname("vst"))
                    nc.vector.tensor_tensor(out=vst[:, :CH], in0=ps[:, :CH],
                                            in1=bv_rep[:], op=ALU.add)
                    nc.vector.tensor_scalar(
                        out=vaug[:, j, :, 0:DV],
                        in0=vst[:, :CH].rearrange("p (h d) -> p h d", h=HPG),
                        scalar1=valid_sb[:, j:j + 1], scalar2=None, op0=ALU.mult)
                    for h in range(HPG):
                        nc.gpsimd.tensor_copy(out=vaug[:, j, h, DV:DV + 1],
                                              in_=valid_bf[:, j:j + 1])

            def push_vproj_fillers(j):
                st = {}
                fillers.append((f"vp{j}", lambda j=j, st=st: vproj_mms(j, 0, 4, st)))
                fillers.append((f"vp{j}", lambda j=j, st=st: vproj_mms(j, 4, DJ, st)))

            # ---- q projection ---------------------------------------------
            def qproj_mms(m, qb, dj0, dj1, st):
                if "ps" not in st:
                    st["ps"] = psC.tile([P, 512], F32, tag="pj", name=uname("qps"))
                for dj in range(dj0, dj1):
                    nc.tensor.matmul(
                        st["ps"][:],
                        lhsT=wq_sb[:, dj, m * P:(m + 1) * P],
                        rhs=xq_t[dj][:, qb * 512:(qb + 1) * 512],
                        start=(dj == 0), stop=(dj == DJ - 1),
                        skip_group_check=True)
                if dj1 == DJ:
                    nc.vector.tensor_scalar(
                        out=qT_sb[:, m, qb * 512:(qb + 1) * 512], in0=st["ps"][:],
                        scalar1=bq_sb[:, m:m + 1], scalar2=None, op0=ALU.add)

            def push_qproj_fillers(m, qb):
                st = {}
                for q in range(4):
                    fillers.append((f"qp{m}{qb}", (lambda m=m, qb=qb, q=q, st=st:
                                                   qproj_mms(m, qb, 2 * q, 2 * q + 2, st))))

            # ---- out-projection (as fillers), per (q-chunk, n-half) -------
            op_stage = {}

            def outproj_piece(qc, n2):
                if qc not in op_stage:
                    op_stage[qc] = op.tile([P, D], BF16, tag="o", name=f"og{qc}")
                stage = op_stage[qc]
                ps = psC.tile([P, 512], F32, tag="pj", name=uname("ops"))
                for m in range(MC):
                    nc.tensor.matmul(
                        ps[:],
                        lhsT=ctxN[:, m, qc * P:(qc + 1) * P],
                        rhs=wo_sb[:, m, n2 * 512:(n2 + 1) * 512],
                        start=(m == 0), stop=(m == MC - 1),
                        skip_group_check=True)
                if qc >= (NQB - 1) * 4:
                    nc.scalar.copy(out=stage[:, n2 * 512:(n2 + 1) * 512], in_=ps[:])
                else:
                    nc.vector.tensor_copy(out=stage[:, n2 * 512:(n2 + 1) * 512], in_=ps[:])
                if n2 == 1:
                    nc.sync.dma_start(out=out.ap()[qc * P:(qc + 1) * P, :], in_=stage[:])
                    del op_stage[qc]

            def push_outproj_fillers(qb):
                for qc in range(qb * 4, qb * 4 + 4):
                    for n2 in range(2):
                        fillers.append(("op", lambda qc=qc, n2=n2: outproj_piece(qc, n2)))

            # ---- attention ------------------------------------------------
            # Pending AV is global so the one-group software skew spans pair
            # boundaries with no ACT gap at the seams.
            pend = {"av": None}

            def emit_av(p, qb, j, ex, ctx_ps, last):
                drain_groups({f"vp{j}"})  # vaug[j] producers must precede
                for hh in range(2):
                    nc.tensor.matmul(
                        ctx_ps[0:DV + 1, hh * 512:hh * 512 + 512],
                        lhsT=vaug[:, j, 2 * p + hh, :],
                        rhs=ex[:, hh * 512:(hh + 1) * 512],
                        start=(j == 0), stop=(j == NJ - 1),
                        skip_group_check=True)
                if last:
                    finish_pair(p, qb, ctx_ps)

            def flush_av():
                if pend["av"] is not None:
                    fn = pend["av"]
                    pend["av"] = None
                    fn()

            def emit_attention(p, qb):
                # safety: inputs of this block must already be emitted
                drain_groups({f"kp{p}", f"qp{p}{qb}"})
                q0 = qb * QB
                ctx_ps = psB.tile([P, 1024], F32, tag="ctx", name=f"ctx{p}{qb}")
                for j in range(NJ):
                    st = psA.tile([P, 1024], F32, tag="st", name=f"st{p}{qb}{j}")
                    for hh in range(2):
                        po = hh * 64
                        nc.tensor.matmul(
                            st[:, hh * 512:(hh + 1) * 512],
                            lhsT=kT_sb[po:po + 64, p, j * P:(j + 1) * P],
                            rhs=qT_sb[po:po + 64, p, q0:q0 + 512],
                            start=True, stop=True)
                    ex = ep.tile([P, 1024], BF16, tag="e", name=f"ex{p}{qb}{j}")
                    nc.scalar.activation(out=ex[:], in_=st[:], func=AF.Exp)
                    k = 2 if (fillers and fillers[0][0][0] in "kvq") else 1
                    if pend["av"] is not None and getattr(pend["av"], "last", False):
                        # release the ctx PSUM buffer ASAP at pair seams
                        flush_av()
                        drain(k)
                    else:
                        drain(k)
                        flush_av()
                    fn = (lambda p=p, qb=qb, j=j, ex=ex, ctx_ps=ctx_ps,
                          last=(j == NJ - 1): emit_av(p, qb, j, ex, ctx_ps, last))
                    fn.last = (j == NJ - 1)
                    pend["av"] = fn

            # ---- finish a (pair, qb): evacuate ctx, normalize -------------
            def finish_pair(p, qb, ctx_ps):
                q0 = qb * QB
                ctxU = cu.tile([P, 1024], F32, tag="cu", name=f"cu{p}{qb}")
                nc.vector.tensor_copy(out=ctxU[0:DV + 1, :], in_=ctx_ps[0:DV + 1, :])
                # reciprocal of the 1024 denominators via a [128, 8] reshape
                # (single-partition DVE reciprocal is ~13us); DRAM bounces
                # do the reshape; all hops on the gpsimd DMA queue.
                rb = dscr.tile([1, 1024], F32, tag="rb")
                nc.gpsimd.dma_start(out=rb[:], in_=ctxU[DV:DV + 1, :])
                rsq = smalls.tile([P, 8], F32, tag="rsq")
                nc.gpsimd.dma_start(out=rsq[:], in_=rb.rearrange("o (p a) -> (o p) a", p=P))
                rcq = smalls.tile([P, 8], F32, tag="rcq")
                nc.vector.reciprocal(out=rcq[:], in_=rsq[:])
                rb2 = dscr.tile([1, 1024], F32, tag="rb2")
                nc.gpsimd.dma_start(out=rb2.rearrange("o (p a) -> (o p) a", p=P), in_=rcq[:])
                recb = scr.tile([P, 1024], F32, tag="s", name=f"rc{p}{qb}")
                nc.gpsimd.dma_start(out=recb[0:64, :],
                                    in_=rb2[0][None, :].partition_broadcast(64))
                # head 2p (even -> partitions 0-63) straight into ctxN
                nc.vector.tensor_tensor(
                    out=ctxN[0:64, p, q0:q0 + QB],
                    in0=ctxU[0:64, 0:512], in1=recb[0:64, 0:512], op=ALU.mult)
                # head 2p+1 (odd -> partitions 64-127) via SB->SB DMA shift
                tmp = scr.tile([P, 1024], BF16, tag="s", name=f"tm{p}{qb}")
                nc.vector.tensor_tensor(
                    out=tmp[0:64, 0:512],
                    in0=ctxU[0:64, 512:1024], in1=recb[0:64, 512:1024], op=ALU.mult)
                nc.sync.dma_start(out=ctxN[64:128, p, q0:q0 + QB], in_=tmp[0:64, 0:512])
                if p == 1:
                    push_outproj_fillers(qb)

            # ---- schedule -------------------------------------------------
            # PE warmup: ~18 dependency-free matmuls keep the PE busy while
            # the first inputs stream in, so HAM grants full clock (K=8/8)
            # before the real projections start instead of ~30us in.
            def warmup(n):
                for _ in range(n):
                    ps = psC.tile([P, 512], F32, tag="pj", name=uname("wrm"))
                    nc.tensor.matmul(ps[:], lhsT=dum[0:P, 0:P], rhs=dum[:],
                                     start=True, stop=True, skip_group_check=True)

            warmup(18)
            kproj_mms(0, 0, 0, DJ, {})
            for j in range(min(2, NJ)):
                vproj_mms(j, 0, DJ, {})
            qproj_mms(0, 0, 0, DJ, {})

            for kb in range(1, NKB):
                st0 = {}
                for q in range(4):
                    fillers.append(("kp0", (lambda kb=kb, q=q, st0=st0:
                                            kproj_mms(0, kb, 2 * q, 2 * q + 2, st0))))
            for j in range(2, NJ):
                push_vproj_fillers(j)
            push_qproj_fillers(0, 1)
            push_kproj_fillers(1)
            push_qproj_fillers(1, 0)
            push_qproj_fillers(1, 1)
            for qb in (2, 3):
                for m in range(MC):
                    push_qproj_fillers(m, qb)
            # attention order: (0,0),(0,1),(1,0),(1,1),(0,2),(0,3),(1,2),(1,3)
            # moves the second-to-last normalize chain ~18us before the
            # stream end, so the tail waits only on the very last one.
            for p, qb in ((0, 0), (0, 1), (1, 0), (1, 1),
                          (0, 2), (0, 3), (1, 2), (1, 3)):
                emit_attention(p, qb)
            flush_av()          # last AV + finish_pair(1, NQB-1)
            warmup(20)          # hold full clock through the last chain
            drain_all()         # out-projection of the last q-block

    nc.compile()
    return nc


def _ensure_axon_hooks():
    """bass_utils imports antenv.axon_hooks when tracing; this image's antenv
    lacks it. Provide it, backed by the ctypes NTFF hook when available."""
    import sys
    import types
    try:
        import antenv.axon_hooks  # noqa: F401
        return
    except ImportError:
        pass
    hook = None
    try:
        from trn_agent_boot.trn_boot import _ntff_profile_via_ctypes
        hook = _ntff_profile_via_ctypes("/opt/axon/libaxon_pjrt.so")
    except Exception:
        hook = None
    mod = types.ModuleType("antenv.axon_hooks")
    mod._hook = hook
    mod.get_axon_ntff_profile_hook = lambda: mod._hook
    mod.set_axon_ntff_profile_hook = lambda h: setattr(mod, "_hook", h)
    sys.modules["antenv.axon_hooks"] = mod


def kernel(Q, K, V, atte_mask_out, Wq, bq, Wk, bk, Wv, bv, Wo, bo):
    import jax  # noqa: F401  (must be imported first so the axon backend registers)
    from concourse.bass_utils import run_bass_kernel_spmd
    global LAST_RESULTS
    _ensure_axon_hooks()

    Q = np.asarray(Q); K = np.asarray(K); V = np.asarray(V)
    mask = np.asarray(atte_mask_out).reshape(B, S)
    Wq = np.asarray(Wq); Wk = np.asarray(Wk); Wv = np.asarray(Wv); Wo = np.asarray(Wo)
    bq = np.asarray(bq); bk = np.asarray(bk); bv = np.asarray(bv); bo = np.asarray(bo)

    keep = [np.flatnonzero(~mask[b]) for b in range(B)]
    n_kp = max(P, max(((len(ix) + P - 1) // P) * P for ix in keep))

    # per-batch packed bf16 tensors
    xqT, xkT, xvT, validv = [], [], [], []
    for b in range(B):
        ix = keep[b]
        xqT.append(np.ascontiguousarray(_bf16(Q[b].T)))
        kk = np.zeros((D, n_kp), np.float32)
        vv = np.zeros((D, n_kp), np.float32)
        kk[:, :len(ix)] = K[b][ix].T
        vv[:, :len(ix)] = V[b][ix].T
        xkT.append(_bf16(kk))
        xvT.append(_bf16(vv))
        va = np.zeros(n_kp, np.float32)
        va[:len(ix)] = 1.0
        validv.append(va)

    in_maps = []
    for c in range(NCORES):
        b, g = c // GROUPS, c % GROUPS
        sl = slice(g * CH, (g + 1) * CH)
        in_maps.append({
            "xq0": np.ascontiguousarray(xqT[b][:, 0:512]),
            "xq1": np.ascontiguousarray(xqT[b][:, 512:1024]),
            "xq23": np.ascontiguousarray(xqT[b][:, 1024:]),
            "xkT": xkT[b], "xvT": xvT[b],
            "wqP": _wpack(_bf16(Wq[sl].T / SCALE), CH),
            "wkP": _wpack(_bf16(Wk[sl].T), CH),
            "wvP": _wpack(_bf16(Wv[sl].T), CH),
            "woP": _wpack(_bf16(Wo[:, sl].T), D),
            "bq": np.ascontiguousarray(bq[sl] / SCALE, np.float32),
            "bk": np.ascontiguousarray(bk[sl], np.float32),
            "bv": np.ascontiguousarray(bv[sl], np.float32),
            "valid": validv[b],
        })

    if n_kp not in _BUILD_CACHE:
        _BUILD_CACHE[n_kp] = _build(n_kp)
    nc = _BUILD_CACHE[n_kp]

    res = run_bass_kernel_spmd(nc, in_maps, core_ids=list(range(NCORES)))
    LAST_RESULTS = res

    full = np.zeros((B, S, D), np.float32)
    full += bo.astype(np.float32)
    for c in range(NCORES):
        full[c // GROUPS] += np.asarray(res.results[c]["out"], np.float32)
    return full
